# revision 5
# baseline (speedup 1.0000x reference)
"""LorentzConv2d Trainium2 kernel v4.

Full-input contract: kernel(x=[8,56,56,64], kernels=[64,64]) -> [8,56,56,64].
Data-parallel over batch: one image per NeuronCore (8 cores).

Per-core algorithm on the zero-padded 58x58 grid, linearized l = 58*gh+gw,
tiled l = 128*t + p (p = partition):
  u[l,o]   = sum_c x[l,c] g_c k[o,c]    (PE, f32; col O accumulates sx)
  D[l,o]   = acosh(u)^2 = ln(u + sqrt(u^2-1+g))^2   (ACT chain, bf16,
             pipelined per tile-group so D tiles emerge early)
  Q[l,o]   = -box3x3(D^2) + 2 sum_d box_d( D_si * D_sj * G_d )   (PE bands)
  S1[l,o]  = box3x3(sx*D)
  out_o    = (S1/63) * exp(-0.5 ln(-Q)) ; out_0 = exp(0.5 ln(1+sum out_o^2))
Shifted fields come from SBUF->SBUF partition-shift DMA copies spread over
the SP and ACT DGE queues (consts ride Pool's SWDGE to dodge the serialized
HWDGE).  A difference basis S={0,1,2,58,60,116,118} expresses every
window-pair offset d as sj-si, so only 13 shifted copies are needed.
Engine balance: DVE keeps the G products/trees and most pair muls; Pool
takes the vv add and the last two deltas' pair muls; ACT absorbs gx16,
x16 tail, and the diag field (Square).
"""

import os
import numpy as np

import concourse.bass as bass
import concourse.bacc as bacc
import concourse.tile as tile
from concourse import mybir
from concourse.bass_utils import run_bass_kernel_spmd

# Restrict activation tables to the two actually used; keeps ln+exp+square in
# one set so the scheduler emits exactly two LoadActFuncSets.
import concourse.bacc as _bacc_mod
from concourse.hw_specs import get_activation_tables as _orig_gat


def _gat(arch):
    # Keep every table at its original index (act_func_set_id is positional
    # in act_info.json), but empty the ones we don't want chosen.
    tabs = _orig_gat(arch)
    keep = {"sqrt_and_others", "natural_log_exp_and_others"}
    if keep <= set(tabs):
        return {k: (v if k in keep else set()) for k, v in tabs.items()}
    return tabs


_bacc_mod.get_activation_tables = _gat

F32 = mybir.dt.float32
BF16 = mybir.dt.bfloat16
AF = mybir.ActivationFunctionType
OP = mybir.AluOpType

# geometry
H = W = 56
C = 64
O = 64
GW = 58                  # padded grid width (58x58)
NT = 27                  # 128-row tiles covering 58*58=3364 (+ tail)
NP = NT * 128            # 3456
NT1 = NT + 1             # +1 zero tail tile for shifted reads
NPADF = 2                # leading zero pad tiles in f fields (sides j>=-2)
NTF = NT + 3             # f field tiles: 2 lead + NT + 1 trail
SQ_GUARD = 1e-4          # replaces the max(u, 1+eps) clamp inside sqrt

# (dh, dw) per positive window-pair offset d = 58*dh + dw
DELTAS = {1: (0, 1), 2: (0, 2), 56: (1, -2), 57: (1, -1), 58: (1, 0),
          59: (1, 1), 60: (1, 2), 114: (2, -2), 115: (2, -1), 116: (2, 0),
          117: (2, 1), 118: (2, 2)}
# d -> (si, sj) with d = sj - si, both in the copy basis
PAIRS = {1: (1, 2), 2: (0, 2), 56: (2, 58), 57: (1, 58), 58: (0, 58),
         59: (1, 60), 60: (0, 60), 114: (2, 116), 115: (1, 116),
         116: (0, 116), 117: (1, 118), 118: (0, 118)}
XSHIFTS = [2, 58, 60, 116, 118]   # unsigned x copies (xc_s)
GXSHIFTS = [1, 2]                 # signed (col0-negated) copies (gxc_s)
DSHIFTS = [1, 2, 58, 60, 116, 118]
# delta processing order: by when source copies are likely ready
DORDER = [2, 1, 58, 57, 56, 60, 59, 116, 115, 114, 118, 117]
POOL_DS = {118, 117}     # deltas whose pair muls run on the Pool engine
# dist-chain tile groups (pipelined): psu PSUM tiles are per-group
UGROUPS = [(0, 7), (7, 7), (14, 7), (21, 6)]


def _interval(d):
    return range(max(-1, -1 - d), min(1, 1 - d) + 1)


def _build_passes():
    """pass = (name, dkey_or_None, coeff, box_offsets(after -si), target).
    Ordered to match field completion order (PE consumes passes in order):
    the first two deltas' fields finish before diag/s1 are emitted."""
    box33 = [58 * a + b for a in (-1, 0, 1) for b in (-1, 0, 1)]

    def dpass(d):
        dh, dw = DELTAS[d]
        si, _ = PAIRS[d]
        box = [58 * a + b - si for a in _interval(dh) for b in _interval(dw)]
        return (f"d{d}", d, 2.0, box, "q")

    passes = [dpass(DORDER[0]), dpass(DORDER[1]),
              ("diag", None, -1.0, box33, "q"),
              ("s1", None, 1.0, box33, "s")]
    for d in DORDER[2:]:
        passes.append(dpass(d))
    return passes


def _build_bands(passes):
    """Banded-Toeplitz matrices. T[m, i] = coeff iff the source row m of tile
    c+j supplies out row i:  m = i + t - 128j for t in box."""
    mats = []
    sides = []
    for (_, _, coeff, box, _) in passes:
        plist = []
        for j in (-2, -1, 0, 1):
            T = np.zeros((128, 128), dtype=np.float32)
            for t in set(box):
                dd = t - 128 * j
                if -127 <= dd <= 127:
                    idx = np.arange(max(0, dd), 128 + min(0, dd))
                    T[idx, idx - dd] = coeff
            if np.any(T):
                plist.append((j, len(mats)))
                mats.append(T)
        sides.append(plist)
    return np.stack(mats), sides


PASSES = _build_passes()
BANDS, PASS_SIDES = _build_bands(PASSES)
NB = BANDS.shape[0]
CHUNKS = [(0, 8), (8, 8), (16, 8), (24, 3)]


def _shift_copy(nc, dst, src, s, eng_a=None, eng_b=None, groups=None):
    """dst[p, 0:NT, :] = src rows l+s (l = 128t+p), via two partition-shifted
    SBUF->SBUF DMAs. src is [128, NT1, inner] with a zero tail tile.
    groups: optional list of (t0, tn) to emit per-piece (earlier starts)."""
    eng_a = eng_a or nc.sync
    eng_b = eng_b or eng_a
    assert 0 < s < 128
    if groups is None:
        groups = [(0, NT)]
    for (t0, tn) in groups:
        eng_a.dma_start(out=dst[0:128 - s, t0:t0 + tn, :],
                        in_=src[s:128, t0:t0 + tn, :])
        eng_b.dma_start(out=dst[128 - s:128, t0:t0 + tn, :],
                        in_=src[0:s, t0 + 1:t0 + tn + 1, :])


def _rep2(t, n_inner):
    """[128, NT, 2] tile viewed as [128, NT, n_inner/2, 2] via paired
    stride-1 reads (keeps the DVE 16-bit 2x mode on broadcast multiplies)."""
    return t[:].unsqueeze(2).to_broadcast([128, NT, n_inner // 2, 2])


def _as4(ap, n_inner):
    """[128, NT, n_inner] AP viewed as [128, NT, n_inner/2, 2]."""
    return ap.rearrange("p t (a b) -> p t a b", b=2)


def build_nc(reps=1):
    nc = bacc.Bacc(None)
    # x arrives host-padded onto the 58x58 grid (+ zero tail tile)
    x_in = nc.declare_dram_parameter("x", [NT1 * 128, C], F32, isOutput=False)
    x16_in = nc.declare_dram_parameter("x16", [NT1 * 128, C], BF16,
                                       isOutput=False)
    gk_in = nc.declare_dram_parameter("gk_ext", [C, O + 1], F32,
                                      isOutput=False)
    # bands stored partition-major on host: [p][band][m]
    bands_in = nc.declare_dram_parameter("bands", [128, NB, 128], BF16,
                                         isOutput=False)
    id_in = nc.declare_dram_parameter("ident", [128, 128], F32, isOutput=False)
    # full padded output field; host extracts the 56x56 interior
    out_ext = nc.declare_dram_parameter("out", [NP, O], F32, isOutput=True)

    with tile.TileContext(nc) as tc:
        for rep in range(reps):
            with (
                tc.tile_pool(name=f"sg{rep}", bufs=1) as sg,
                tc.tile_pool(name=f"pp{rep}", bufs=1) as pp,
            ):
                _one_rep(nc, tc, sg, pp, x_in, x16_in, gk_in,
                         bands_in, id_in, out_ext, rep)
    nc.finalize()
    return nc


def _one_rep(nc, tc, sg, pp, x_in, x16_in, gk_in, bands_in, id_in,
             out_ext, rep):
    r = f"r{rep}_"

    def T(shape, dt, name):
        return sg.tile(shape, dt, tag=r + name, name=r + name)

    # ---- constants ride Pool's SWDGE: keeps the serialized HWDGE free for
    # the x load and the shift-copy stream.  id first (gates transposes).
    id_sb = T([128, 128], F32, "id")
    nc.gpsimd.dma_start(out=id_sb[:], in_=id_in[:])
    gk_sb = T([C, O + 1], F32, "gk")
    nc.gpsimd.dma_start(out=gk_sb[:], in_=gk_in[:])

    # ---- x f32 chunks on SP; x16 bf16 on SP early (feeds shift copies)
    x_sb = T([128, NT1, C], F32, "x_sb")
    xview = x_in.rearrange("(t p) c -> p t c", p=128)
    x16 = T([128, NT1, C], BF16, "x16")
    nc.sync.dma_start(out=x16[:],
                      in_=x16_in.rearrange("(t p) c -> p t c", p=128))
    for (t0, tn) in UGROUPS:
        nc.sync.dma_start(out=x_sb[:, t0:t0 + tn + (1 if t0 + tn == NT else 0), :],
                          in_=xview[:, t0:t0 + tn + (1 if t0 + tn == NT else 0), :])

    # gx16 (col0-negated x16) on ACT: idle early, frees DVE
    gx16 = T([128, NT1, C], BF16, "gx16")
    nc.scalar.activation(gx16[:, :, 1:C], x16[:, :, 1:C], AF.Copy)
    nc.scalar.activation(gx16[:, :, 0:1], x16[:, :, 0:1], AF.Copy, scale=-1.0)

    # ---- x/gx shift copies: xc[2] first (feeds d=2 with gxc[0]=gx16),
    # spread across SP and ACT DGE queues
    xc = {0: x16}
    gxc = {0: gx16}
    shift_jobs = ([("x", 2), ("gx", 1), ("gx", 2)]
                  + [("x", s) for s in XSHIFTS if s != 2])
    for i, (kind, s) in enumerate(shift_jobs):
        src, dstmap = (x16, xc) if kind == "x" else (gx16, gxc)
        dstmap[s] = T([128, NT, C], BF16, f"{kind}c{s}")
        eng = nc.sync if i % 2 == 0 else nc.scalar
        _shift_copy(nc, dstmap[s], src, s, eng_a=eng, eng_b=eng)

    bands_sb = T([128, NB, 128], BF16, "bands")
    nc.gpsimd.dma_start(out=bands_sb[:], in_=bands_in[:])

    # ---- phase A: transposes + u matmuls + per-group dist chain.
    # Chain per tile-group so D tiles emerge early and the D-shift DMAs /
    # pair muls overlap the later groups' chain.
    d16 = T([128, NT1, O], BF16, "d16")
    nc.vector.memset(d16[:, NT, :], 0.0)
    sx_sb = T([128, NT], F32, "sx")
    cm1g = T([128, 1], F32, "cm1g")
    nc.gpsimd.memset(cm1g[:], -1.0 + SQ_GUARD)
    cmone = T([128, 1], F32, "cmone")
    nc.gpsimd.memset(cmone[:], -1.0)
    clnb = T([128, 1], F32, "clnb")
    nc.gpsimd.memset(clnb[:], 1e-30)
    sx2 = T([128, NT, 2], BF16, "sx2")

    with (
        tc.tile_pool(name=r + "psA", bufs=1, space="PSUM") as psA,
        tc.tile_pool(name=r + "psT", bufs=3, space="PSUM") as psT,
    ):
        xT = T([64, NT, 128], F32, "xT")
        psu_g = [psA.tile([128, 7, O + 1], F32, tag=f"{r}psu{i}",
                          name=f"{r}psu{i}") for i in range(4)]
        for gi, (t0, tn) in enumerate(UGROUPS):
            for i in range(tn):
                tl = t0 + i
                xt_ps = psT.tile([C, 128], F32)
                nc.tensor.transpose(xt_ps[:], x_sb[:, tl, :], id_sb[:])
                nc.scalar.copy(xT[:, tl, :], xt_ps[:])
                nc.tensor.matmul(psu_g[gi][:, i, :], xT[:, tl, :], gk_sb[:],
                                 start=True, stop=True)

        # per-group dist chain:
        #   sq = u^2 ; rl1 = relu(sq-1+g) ; rt = sqrt(rl1)
        #   vv = u + rt - 1   (Pool STT)
        #   D  = ln(1 + relu(vv))^2  (relu zeroes the all-zero pad rows)
        for gi, (t0, tn) in enumerate(UGROUPS):
            b = f"g{gi % 2}_"
            u_ps = psu_g[gi][:, :tn, 0:O]
            sq = pp.tile([128, 7, O], F32, tag=b + "sq", name=f"{r}sq{gi}")
            nc.scalar.activation(sq[:, :tn, :], u_ps, AF.Square)
            # Pool can't read PSUM: stage u into SBUF for the Pool vv add
            u_sb = pp.tile([128, 7, O], F32, tag=b + "u", name=f"{r}u{gi}")
            nc.scalar.copy(u_sb[:, :tn, :], u_ps)
            nc.scalar.copy(sx_sb[:, t0:t0 + tn], psu_g[gi][:, :tn, O])
            rl1 = pp.tile([128, 7, O], F32, tag=b + "rl1", name=f"{r}rl1{gi}")
            nc.scalar.activation(rl1[:, :tn, :], sq[:, :tn, :], AF.Relu,
                                 bias=cm1g[:])
            rt = pp.tile([128, 7, O], F32, tag=b + "rt", name=f"{r}rt{gi}")
            nc.scalar.activation(rt[:, :tn, :], rl1[:, :tn, :], AF.Sqrt)
            # vv = u + rt (plain add -> legal on Pool); the -1 folds into the
            # Relu bias: rl = relu(u + rt - 1); lnv = ln(1 + rl)
            vv = pp.tile([128, 7, O], F32, tag=b + "sq", name=f"{r}vv{gi}")
            nc.gpsimd.tensor_add(vv[:, :tn, :], u_sb[:, :tn, :],
                                 rt[:, :tn, :])
            rl = pp.tile([128, 7, O], F32, tag=b + "rl1", name=f"{r}rl{gi}")
            nc.scalar.activation(rl[:, :tn, :], vv[:, :tn, :], AF.Relu,
                                 bias=cmone[:])
            lnv = pp.tile([128, 7, O], F32, tag=b + "sq", name=f"{r}lnv{gi}")
            nc.scalar.activation(lnv[:, :tn, :], rl[:, :tn, :], AF.Ln,
                                 bias=1.0)
            nc.scalar.activation(d16[:, t0:t0 + tn, :], lnv[:, :tn, :],
                                 AF.Square)
        nc.gpsimd.tensor_copy(sx2[:], sx_sb[:].unsqueeze(2).to_broadcast(
            [128, NT, 2]))

    # ---- shifted D copies: s=1,2 split in halves (earlier starts, feed the
    # first pair muls); long shifts full-tensor.  Spread SP/ACT queues.
    HALVES = [(0, 14), (14, 13)]
    dc = {0: d16}
    for i, s in enumerate(DSHIFTS):
        dc[s] = T([128, NT, O], BF16, f"dc{s}")
        eng = nc.sync if i % 2 == 0 else nc.scalar
        groups = HALVES if s in (1, 2) else None
        _shift_copy(nc, dc[s], d16, s, eng_a=eng, eng_b=eng, groups=groups)

    # ---- fields: per-delta pipeline with G-product lookahead.  DVE does the
    # G products/trees and most pair muls; Pool takes POOL_DS' pair muls;
    # ACT takes the diag field (Square of d16).
    fields = {}

    def new_field(key):
        f = sg.tile([128, NTF, O], BF16, tag=f"{r}f{key}", name=f"{r}f{key}")
        nc.gpsimd.memset(f[:, 0:NPADF, :], 0.0)
        nc.gpsimd.memset(f[:, NPADF + NT:, :], 0.0)
        fields[key] = f
        return f

    g2 = {}
    tg = {}
    LOOK = 3

    def emit_prod(i):
        d = DORDER[i]
        si, sj = PAIRS[d]
        t = pp.tile([128, NT, C], BF16, tag=f"tg{i % (LOOK + 1)}",
                    name=f"{r}tg{d}")
        nc.vector.tensor_mul(t[:], gxc[si][:, 0:NT, :], xc[sj][:, 0:NT, :])
        tg[d] = t

    for i in range(LOOK):
        emit_prod(i)
    for i, d in enumerate(DORDER):
        if i + LOOK < len(DORDER):
            emit_prod(i + LOOK)
        t = tg[d]
        w = C // 2
        lvl = 0
        while w >= 2:  # bf16 halving levels on DVE (2x mode)
            o_t = pp.tile([128, NT, w], BF16, tag=f"tr{i % 2}_{lvl}",
                          name=f"{r}tr{d}_{lvl}")
            with nc.allow_low_precision(reason="bf16 tree partials"):
                nc.vector.tensor_add(o_t[:], t[:, :, 0:w], t[:, :, w:2 * w])
            t = o_t
            w //= 2
            lvl += 1
        g = T([128, NT, 2], BF16, f"g{d}")
        with nc.allow_low_precision(reason="bf16 G"):
            nc.vector.tensor_add(
                g[:], t[:, :, 0:1].to_broadcast([128, NT, 2]),
                t[:, :, 1:2].to_broadcast([128, NT, 2]))
        g2[d] = g
        si, sj = PAIRS[d]
        f = new_field(f"d{d}")
        if d in POOL_DS:
            t2 = sg.tile([128, NT, O], BF16, tag=f"{r}t2p{d}",
                         name=f"{r}t2p{d}")
            nc.gpsimd.tensor_mul(t2[:], dc[si][:, 0:NT, :],
                                 dc[sj][:, 0:NT, :])
            with nc.allow_low_precision(reason="bf16 field"):
                nc.gpsimd.tensor_mul(
                    f[:, NPADF:NPADF + NT, :], t2[:],
                    g2[d][:, :, 0:1].to_broadcast([128, NT, O]))
        else:
            t2 = pp.tile([128, NT, O], BF16, tag=f"t2{i % 2}",
                         name=f"{r}t2{d}")
            nc.vector.tensor_mul(t2[:], dc[si][:, 0:NT, :],
                                 dc[sj][:, 0:NT, :])
            nc.vector.tensor_mul(_as4(f[:, NPADF:NPADF + NT, :], O),
                                 _as4(t2[:], O), _rep2(g2[d], O))
        if i == 1:
            fdiag = new_field("diag")
            nc.scalar.activation(fdiag[:, NPADF:NPADF + NT, :],
                                 d16[:, 0:NT, :], AF.Square)
            fs1 = new_field("s1")
            nc.vector.tensor_mul(_as4(fs1[:, NPADF:NPADF + NT, :], O),
                                 _as4(d16[:, 0:NT, :], O), _rep2(sx2, O))

    # ---- pass-major banded box matmuls (all 4 chunks' PSUM live)
    osb = T([128, NT, O], F32, "osb")
    with (
        tc.tile_pool(name=r + "psQ", bufs=1, space="PSUM") as psQ,
        tc.tile_pool(name=r + "psS", bufs=1, space="PSUM") as psS,
    ):
        ps_q = [psQ.tile([128, cw, O], F32, tag=f"{r}psq{ci}",
                         name=f"{r}psq{ci}") for ci, (c0, cw) in
                enumerate(CHUNKS)]
        ps_s = [psS.tile([128, cw, O], F32, tag=f"{r}pss{ci}",
                         name=f"{r}pss{ci}") for ci, (c0, cw) in
                enumerate(CHUNKS)]
        wq = [0] * len(CHUNKS)
        ws = [0] * len(CHUNKS)
        nwq = sum(len(PASS_SIDES[pi]) for pi, p in enumerate(PASSES)
                  if p[4] == "q")
        nws = sum(len(PASS_SIDES[pi]) for pi, p in enumerate(PASSES)
                  if p[4] == "s")

        def box_half(chunk_ids):
            """Pass-major over a half of the chunks: PE consumes each field
            as soon as it is built; the other half's phase D overlaps."""
            for pi, (pname, dkey, coeff, box, tgt_kind) in enumerate(PASSES):
                fkey = "diag" if pname == "diag" else (
                    "s1" if pname == "s1" else f"d{dkey}")
                f = fields[fkey]
                for (j, bi) in PASS_SIDES[pi]:
                    for ci in chunk_ids:
                        c0, cw = CHUNKS[ci]
                        if tgt_kind == "q":
                            tgt, first, last = ps_q[ci], wq[ci] == 0, \
                                wq[ci] == nwq - 1
                            wq[ci] += 1
                        else:
                            tgt, first, last = ps_s[ci], ws[ci] == 0, \
                                ws[ci] == nws - 1
                            ws[ci] += 1
                        nc.tensor.matmul(
                            tgt[:], bands_sb[:, bi, :],
                            f[:, NPADF + c0 + j:NPADF + c0 + j + cw, :],
                            start=first, stop=last, skip_group_check=True)

        # ---- normalize/emit per chunk: rr = (-Q)^-1/2 via ln+exp
        def phase_d(ci):
            c0, cw = CHUNKS[ci]
            lnq = pp.tile([128, cw, O], F32, tag=f"lnq{ci % 2}",
                          name=f"{r}lnq{ci}")
            nc.scalar.activation(lnq[:], ps_q[ci][:], AF.Ln, scale=-1.0,
                                 bias=clnb[:])
            rr = pp.tile([128, cw, O], F32, tag=f"rr{ci % 2}",
                         name=f"{r}rr{ci}")
            nc.scalar.activation(rr[:], lnq[:], AF.Exp, scale=-0.5)
            nc.vector.scalar_tensor_tensor(
                out=osb[:, c0:c0 + cw, :], in0=ps_s[ci][:],
                scalar=1.0 / 63.0, in1=rr[:], op0=OP.mult, op1=OP.mult)
            s2 = pp.tile([128, cw, O - 1], F32, tag=f"s2{ci % 2}",
                         name=f"{r}s2{ci}")
            nc.scalar.activation(s2[:], osb[:, c0:c0 + cw, 1:O], AF.Square)
            red = pp.tile([128, cw], F32, tag=f"red{ci % 2}",
                          name=f"{r}red{ci}")
            nc.vector.tensor_reduce(red[:], s2[:], axis=mybir.AxisListType.X,
                                    op=OP.add)
            ln0 = pp.tile([128, cw], F32, tag=f"ln0{ci % 2}",
                          name=f"{r}ln0{ci}")
            nc.scalar.activation(ln0[:], red[:], AF.Ln, bias=1.0)
            nc.scalar.activation(osb[:, c0:c0 + cw, 0], ln0[:], AF.Exp,
                                 scale=0.5)
            oview = out_ext[128 * c0:128 * (c0 + cw), :].rearrange(
                "(t p) c -> p t c", p=128)
            # cols 1..63 are final after the stt; ship them while the out0
            # subchain (s2/red/ln0/exp0) still runs, then the tiny col-0 DMA
            eng = nc.sync if ci % 2 == 0 else nc.scalar
            eng.dma_start(out=oview[:, :, 1:O], in_=osb[:, c0:c0 + cw, 1:O])
            eng.dma_start(out=oview[:, :, 0:1], in_=osb[:, c0:c0 + cw, 0:1])

        box_half([0, 1])
        phase_d(0)
        box_half([2])
        phase_d(1)
        box_half([3])
        phase_d(2)
        phase_d(3)


_NC_CACHE = None


def _get_nc():
    global _NC_CACHE
    if _NC_CACHE is None:
        _NC_CACHE = build_nc()
    return _NC_CACHE


def host_consts(kernels):
    # u = -l_inner(x,k) = x0*k0 - sum_{c>=1} x_c*k_c ; col O is sum_{c>=1} x_c
    gk_ext = np.zeros((C, O + 1), dtype=np.float32)
    gk_ext[:, :O] = kernels.astype(np.float32).T
    gk_ext[1:, :O] *= -1.0
    gk_ext[1:, O] = 1.0
    return gk_ext


def pad_image(img):
    """[56,56,64] -> host-padded [NT1*128, 64] on the 58x58 grid."""
    xp = np.zeros((NT1 * 128, C), dtype=np.float32)
    grid = xp[:GW * GW].reshape(GW, GW, C)
    grid[1:57, 1:57] = img
    return xp


def unpad_out(o):
    """[NP,64] padded field -> [56,56,64] interior."""
    return o[:GW * GW].reshape(GW, GW, O)[1:57, 1:57]


def core_inputs(x, kernels, core=0):
    import ml_dtypes
    xp = pad_image(np.asarray(x[core], dtype=np.float32))
    x16 = xp.astype(ml_dtypes.bfloat16)
    return {
        "x": xp,
        "x16": x16,
        "gk_ext": np.ascontiguousarray(host_consts(kernels)),
        "bands": np.ascontiguousarray(
            BANDS.transpose(1, 0, 2).astype(ml_dtypes.bfloat16)),
        "ident": np.eye(128, dtype=np.float32),
    }


def kernel(x, kernels):
    x = np.asarray(x, dtype=np.float32)
    kernels = np.asarray(kernels, dtype=np.float32)
    B = x.shape[0]
    assert x.shape == (B, H, W, C) and B == 8, x.shape
    nc = _get_nc()
    in_maps = [core_inputs(x, kernels, core=i) for i in range(8)]
    res = run_bass_kernel_spmd(nc, in_maps, core_ids=list(range(8)),
                               trace=bool(int(os.environ.get("KTRACE", "0"))))
    if res.exec_time_ns is not None:
        print(f"HW exec time: {res.exec_time_ns} ns")
    out = np.stack([unpad_out(res.results[i]["out"]) for i in range(8)])
    return out.astype(np.float32)


# revision 8
# speedup vs baseline: 1.0543x; 1.0543x over previous
"""LorentzConv2d Trainium2 kernel v4.

Full-input contract: kernel(x=[8,56,56,64], kernels=[64,64]) -> [8,56,56,64].
Data-parallel over batch: one image per NeuronCore (8 cores).

Per-core algorithm on the zero-padded 58x58 grid, linearized l = 58*gh+gw,
tiled l = 128*t + p (p = partition):
  u[l,o]   = sum_c x[l,c] g_c k[o,c]    (PE, f32; col O accumulates sx)
  D[l,o]   = acosh(u)^2 = ln(u + sqrt(u^2-1+g))^2   (ACT chain, bf16,
             pipelined per tile-group so D tiles emerge early)
  Q[l,o]   = -box3x3(D^2) + 2 sum_d box_d( D_si * D_sj * G_d )   (PE bands)
  S1[l,o]  = box3x3(sx*D)
  out_o    = (S1/63) * exp(-0.5 ln(-Q)) ; out_0 = exp(0.5 ln(1+sum out_o^2))
Shifted fields come from SBUF->SBUF partition-shift DMA copies spread over
the SP and ACT DGE queues (consts ride Pool's SWDGE to dodge the serialized
HWDGE).  A difference basis S={0,1,2,58,60,116,118} expresses every
window-pair offset d as sj-si, so only 13 shifted copies are needed.
Engine balance: DVE keeps the G products/trees and most pair muls; Pool
takes the vv add and the last two deltas' pair muls; ACT absorbs gx16,
x16 tail, and the diag field (Square).
"""

import os
import numpy as np

import concourse.bass as bass
import concourse.bacc as bacc
import concourse.tile as tile
from concourse import mybir
from concourse.bass_utils import run_bass_kernel_spmd

# Restrict activation tables to the two actually used; keeps ln+exp+square in
# one set so the scheduler emits exactly two LoadActFuncSets.
import concourse.bacc as _bacc_mod
from concourse.hw_specs import get_activation_tables as _orig_gat


def _gat(arch):
    # Keep every table at its original index (act_func_set_id is positional
    # in act_info.json), but empty the ones we don't want chosen.
    tabs = _orig_gat(arch)
    keep = {"sqrt_and_others", "natural_log_exp_and_others"}
    if keep <= set(tabs):
        return {k: (v if k in keep else set()) for k, v in tabs.items()}
    return tabs


_bacc_mod.get_activation_tables = _gat

F32 = mybir.dt.float32
BF16 = mybir.dt.bfloat16
AF = mybir.ActivationFunctionType
OP = mybir.AluOpType

# geometry
H = W = 56
C = 64
O = 64
GW = 58                  # padded grid width (58x58)
NT = 27                  # 128-row tiles covering 58*58=3364 (+ tail)
NP = NT * 128            # 3456
NT1 = NT + 1             # +1 zero tail tile for shifted reads
NPADF = 2                # leading zero pad tiles in f fields (sides j>=-2)
NTF = NT + 3             # f field tiles: 2 lead + NT + 1 trail
SQ_GUARD = 1e-4          # replaces the max(u, 1+eps) clamp inside sqrt

# (dh, dw) per positive window-pair offset d = 58*dh + dw
DELTAS = {1: (0, 1), 2: (0, 2), 56: (1, -2), 57: (1, -1), 58: (1, 0),
          59: (1, 1), 60: (1, 2), 114: (2, -2), 115: (2, -1), 116: (2, 0),
          117: (2, 1), 118: (2, 2)}
# d -> (si, sj) with d = sj - si, both in the copy basis
PAIRS = {1: (1, 2), 2: (0, 2), 56: (2, 58), 57: (1, 58), 58: (0, 58),
         59: (1, 60), 60: (0, 60), 114: (2, 116), 115: (1, 116),
         116: (0, 116), 117: (1, 118), 118: (0, 118)}
XSHIFTS = [2, 58, 60, 116, 118]   # unsigned x copies (xc_s)
GXSHIFTS = [1, 2]                 # signed (col0-negated) copies (gxc_s)
DSHIFTS = [1, 2, 58, 60, 116, 118]
# delta processing order: by when source copies are likely ready
DORDER = [2, 1, 58, 57, 56, 60, 59, 116, 115, 114, 118, 117]
POOL_DS = {118, 117}     # deltas whose pair muls run on the Pool engine
# dist-chain tile groups (pipelined): psu PSUM tiles are per-group
UGROUPS = [(0, 7), (7, 7), (14, 7), (21, 6)]


def _interval(d):
    return range(max(-1, -1 - d), min(1, 1 - d) + 1)


def _build_passes():
    """pass = (name, dkey_or_None, coeff, box_offsets(after -si), target).
    Ordered to match field completion order (PE consumes passes in order):
    the first two deltas' fields finish before diag/s1 are emitted."""
    box33 = [58 * a + b for a in (-1, 0, 1) for b in (-1, 0, 1)]

    def dpass(d):
        dh, dw = DELTAS[d]
        si, _ = PAIRS[d]
        box = [58 * a + b - si for a in _interval(dh) for b in _interval(dw)]
        return (f"d{d}", d, 2.0, box, "q")

    passes = [dpass(DORDER[0]), dpass(DORDER[1]),
              ("diag", None, -1.0, box33, "q"),
              ("s1", None, 1.0, box33, "s")]
    for d in DORDER[2:]:
        passes.append(dpass(d))
    return passes


def _build_bands(passes):
    """Banded-Toeplitz matrices. T[m, i] = coeff iff the source row m of tile
    c+j supplies out row i:  m = i + t - 128j for t in box."""
    mats = []
    sides = []
    for (_, _, coeff, box, _) in passes:
        plist = []
        for j in (-2, -1, 0, 1):
            T = np.zeros((128, 128), dtype=np.float32)
            for t in set(box):
                dd = t - 128 * j
                if -127 <= dd <= 127:
                    idx = np.arange(max(0, dd), 128 + min(0, dd))
                    T[idx, idx - dd] = coeff
            if np.any(T):
                plist.append((j, len(mats)))
                mats.append(T)
        sides.append(plist)
    return np.stack(mats), sides


PASSES = _build_passes()
BANDS, PASS_SIDES = _build_bands(PASSES)
NB = BANDS.shape[0]
CHUNKS = [(0, 8), (8, 8), (16, 8), (24, 3)]


def _shift_copy(nc, dst, src, s, eng_a=None, eng_b=None, groups=None):
    """dst[p, 0:NT, :] = src rows l+s (l = 128t+p), via two partition-shifted
    SBUF->SBUF DMAs. src is [128, NT1, inner] with a zero tail tile.
    groups: optional list of (t0, tn) to emit per-piece (earlier starts)."""
    eng_a = eng_a or nc.sync
    eng_b = eng_b or eng_a
    assert 0 < s < 128
    if groups is None:
        groups = [(0, NT)]
    for (t0, tn) in groups:
        eng_a.dma_start(out=dst[0:128 - s, t0:t0 + tn, :],
                        in_=src[s:128, t0:t0 + tn, :])
        eng_b.dma_start(out=dst[128 - s:128, t0:t0 + tn, :],
                        in_=src[0:s, t0 + 1:t0 + tn + 1, :])


def _rep2(t, n_inner):
    """[128, NT, 2] tile viewed as [128, NT, n_inner/2, 2] via paired
    stride-1 reads (keeps the DVE 16-bit 2x mode on broadcast multiplies)."""
    return t[:].unsqueeze(2).to_broadcast([128, NT, n_inner // 2, 2])


def _as4(ap, n_inner):
    """[128, NT, n_inner] AP viewed as [128, NT, n_inner/2, 2]."""
    return ap.rearrange("p t (a b) -> p t a b", b=2)


def build_nc(reps=1):
    nc = bacc.Bacc(None)
    # x arrives host-padded onto the 58x58 grid (+ zero tail tile)
    x_in = nc.declare_dram_parameter("x", [NT1 * 128, C], F32, isOutput=False)
    x16_in = nc.declare_dram_parameter("x16", [NT1 * 128, C], BF16,
                                       isOutput=False)
    gk_in = nc.declare_dram_parameter("gk_ext", [C, O + 1], F32,
                                      isOutput=False)
    # bands stored partition-major on host: [p][band][m]
    bands_in = nc.declare_dram_parameter("bands", [128, NB, 128], BF16,
                                         isOutput=False)
    id_in = nc.declare_dram_parameter("ident", [128, 128], F32, isOutput=False)
    # full padded output field; host extracts the 56x56 interior
    out_ext = nc.declare_dram_parameter("out", [NP, O], F32, isOutput=True)

    with tile.TileContext(nc) as tc:
        for rep in range(reps):
            with (
                tc.tile_pool(name=f"sg{rep}", bufs=1) as sg,
                tc.tile_pool(name=f"pp{rep}", bufs=1) as pp,
            ):
                _one_rep(nc, tc, sg, pp, x_in, x16_in, gk_in,
                         bands_in, id_in, out_ext, rep)
    nc.finalize()
    return nc


def _one_rep(nc, tc, sg, pp, x_in, x16_in, gk_in, bands_in, id_in,
             out_ext, rep):
    r = f"r{rep}_"

    def T(shape, dt, name):
        return sg.tile(shape, dt, tag=r + name, name=r + name)

    # ---- constants ride Pool's SWDGE: keeps the serialized HWDGE free for
    # the x load and the shift-copy stream.  id first (gates transposes).
    id_sb = T([128, 128], F32, "id")
    nc.gpsimd.dma_start(out=id_sb[:], in_=id_in[:])
    gk_sb = T([C, O + 1], F32, "gk")
    nc.gpsimd.dma_start(out=gk_sb[:], in_=gk_in[:])

    # ---- x f32 chunks on SP; chunk 0 first (gates transposes), then the
    # bf16 copy (feeds shift copies), then the remaining chunks
    x_sb = T([128, NT1, C], F32, "x_sb")
    xview = x_in.rearrange("(t p) c -> p t c", p=128)
    x16 = T([128, NT1, C], BF16, "x16")

    def _xchunk(gi):
        t0, tn = UGROUPS[gi]
        tn += 1 if t0 + tn == NT else 0
        nc.sync.dma_start(out=x_sb[:, t0:t0 + tn, :],
                          in_=xview[:, t0:t0 + tn, :])

    _xchunk(0)
    nc.sync.dma_start(out=x16[:],
                      in_=x16_in.rearrange("(t p) c -> p t c", p=128))
    _xchunk(1)
    _xchunk(2)
    _xchunk(3)

    # gx16 (col0-negated x16) on ACT: idle early, frees DVE
    gx16 = T([128, NT1, C], BF16, "gx16")
    nc.scalar.activation(gx16[:, :, 1:C], x16[:, :, 1:C], AF.Copy)
    nc.scalar.activation(gx16[:, :, 0:1], x16[:, :, 0:1], AF.Copy, scale=-1.0)

    # ---- x/gx shift copies: xc[2] first (feeds d=2 with gxc[0]=gx16).
    # All on the SP queue: it is idle after the x load, while the ACT queue
    # would serialize these behind the latency-critical dist chain.
    xc = {0: x16}
    gxc = {0: gx16}
    shift_jobs = ([("x", 2), ("gx", 1), ("gx", 2)]
                  + [("x", s) for s in XSHIFTS if s != 2])
    for i, (kind, s) in enumerate(shift_jobs):
        src, dstmap = (x16, xc) if kind == "x" else (gx16, gxc)
        dstmap[s] = T([128, NT, C], BF16, f"{kind}c{s}")
        _shift_copy(nc, dstmap[s], src, s, eng_a=nc.sync, eng_b=nc.sync)

    bands_sb = T([128, NB, 128], BF16, "bands")
    nc.gpsimd.dma_start(out=bands_sb[:], in_=bands_in[:])

    # ---- phase A: transposes + u matmuls + per-group dist chain.
    # Chain per tile-group so D tiles emerge early and the D-shift DMAs /
    # pair muls overlap the later groups' chain.
    d16 = T([128, NT1, O], BF16, "d16")
    nc.vector.memset(d16[:, NT, :], 0.0)
    sx_sb = T([128, NT], F32, "sx")
    cm1g = T([128, 1], F32, "cm1g")
    nc.gpsimd.memset(cm1g[:], -1.0 + SQ_GUARD)
    cmone = T([128, 1], F32, "cmone")
    nc.gpsimd.memset(cmone[:], -1.0)
    clnb = T([128, 1], F32, "clnb")
    nc.gpsimd.memset(clnb[:], 1e-30)
    sx2 = T([128, NT, 2], BF16, "sx2")

    with (
        tc.tile_pool(name=r + "psA", bufs=1, space="PSUM") as psA,
        tc.tile_pool(name=r + "psT", bufs=3, space="PSUM") as psT,
    ):
        xT = T([64, NT, 128], F32, "xT")
        psu_g = [psA.tile([128, 7, O + 1], F32, tag=f"{r}psu{i}",
                          name=f"{r}psu{i}") for i in range(4)]
        for gi, (t0, tn) in enumerate(UGROUPS):
            for i in range(tn):
                tl = t0 + i
                xt_ps = psT.tile([C, 128], F32)
                nc.tensor.transpose(xt_ps[:], x_sb[:, tl, :], id_sb[:])
                nc.scalar.copy(xT[:, tl, :], xt_ps[:])
                nc.tensor.matmul(psu_g[gi][:, i, :], xT[:, tl, :], gk_sb[:],
                                 start=True, stop=True)

        # per-group dist chain, ALL in the ln/exp act-table set (sqrt is
        # exp(0.5*ln(.)) so no table reloads ever happen):
        #   sq = u^2 ; rl1 = relu(sq-1+g) ; rt = exp(0.5*ln(rl1+eps))
        #   vv = u + rt   (Pool add; u staged to SBUF since Pool can't PSUM)
        #   D  = ln(1 + relu(vv-1))^2  (relu zeroes the all-zero pad rows)
        for gi, (t0, tn) in enumerate(UGROUPS):
            b = f"g{gi % 2}_"
            u_ps = psu_g[gi][:, :tn, 0:O]
            sq = pp.tile([128, 7, O], F32, tag=b + "sq", name=f"{r}sq{gi}")
            nc.scalar.activation(sq[:, :tn, :], u_ps, AF.Square)
            u_sb = pp.tile([128, 7, O], F32, tag=b + "u", name=f"{r}u{gi}")
            nc.scalar.copy(u_sb[:, :tn, :], u_ps)
            nc.scalar.copy(sx_sb[:, t0:t0 + tn], psu_g[gi][:, :tn, O])
            rl1 = pp.tile([128, 7, O], F32, tag=b + "rl1", name=f"{r}rl1{gi}")
            nc.scalar.activation(rl1[:, :tn, :], sq[:, :tn, :], AF.Relu,
                                 bias=cm1g[:])
            lnr = pp.tile([128, 7, O], F32, tag=b + "sq", name=f"{r}lnr{gi}")
            nc.scalar.activation(lnr[:, :tn, :], rl1[:, :tn, :], AF.Ln,
                                 bias=clnb[:])
            rt = pp.tile([128, 7, O], F32, tag=b + "rl1", name=f"{r}rt{gi}")
            nc.scalar.activation(rt[:, :tn, :], lnr[:, :tn, :], AF.Exp,
                                 scale=0.5)
            vv = pp.tile([128, 7, O], F32, tag=b + "sq", name=f"{r}vv{gi}")
            nc.gpsimd.tensor_add(vv[:, :tn, :], u_sb[:, :tn, :],
                                 rt[:, :tn, :])
            rl = pp.tile([128, 7, O], F32, tag=b + "rl1", name=f"{r}rl{gi}")
            nc.scalar.activation(rl[:, :tn, :], vv[:, :tn, :], AF.Relu,
                                 bias=cmone[:])
            lnv = pp.tile([128, 7, O], F32, tag=b + "sq", name=f"{r}lnv{gi}")
            nc.scalar.activation(lnv[:, :tn, :], rl[:, :tn, :], AF.Ln,
                                 bias=1.0)
            nc.scalar.activation(d16[:, t0:t0 + tn, :], lnv[:, :tn, :],
                                 AF.Square)
        nc.gpsimd.tensor_copy(sx2[:], sx_sb[:].unsqueeze(2).to_broadcast(
            [128, NT, 2]))

    # ---- shifted D copies: s=1,2 split in halves (earlier starts, feed the
    # first pair muls); long shifts full-tensor.  Spread SP/ACT queues.
    HALVES = [(0, 14), (14, 13)]
    dc = {0: d16}
    for i, s in enumerate(DSHIFTS):
        dc[s] = T([128, NT, O], BF16, f"dc{s}")
        eng = nc.sync if i % 2 == 0 else nc.scalar
        groups = HALVES if s in (1, 2) else None
        _shift_copy(nc, dc[s], d16, s, eng_a=eng, eng_b=eng, groups=groups)

    # ---- fields: per-delta pipeline with G-product lookahead.  DVE does the
    # G products/trees and most pair muls; Pool takes POOL_DS' pair muls;
    # ACT takes the diag field (Square of d16).
    fields = {}

    def new_field(key):
        f = sg.tile([128, NTF, O], BF16, tag=f"{r}f{key}", name=f"{r}f{key}")
        nc.gpsimd.memset(f[:, 0:NPADF, :], 0.0)
        nc.gpsimd.memset(f[:, NPADF + NT:, :], 0.0)
        fields[key] = f
        return f

    g2 = {}
    tg = {}
    LOOK = 3

    def emit_prod(i):
        d = DORDER[i]
        si, sj = PAIRS[d]
        t = pp.tile([128, NT, C], BF16, tag=f"tg{i % (LOOK + 1)}",
                    name=f"{r}tg{d}")
        nc.vector.tensor_mul(t[:], gxc[si][:, 0:NT, :], xc[sj][:, 0:NT, :])
        tg[d] = t

    for i in range(LOOK):
        emit_prod(i)
    for i, d in enumerate(DORDER):
        if i + LOOK < len(DORDER):
            emit_prod(i + LOOK)
        t = tg[d]
        w = C // 2
        lvl = 0
        while w >= 2:  # bf16 halving levels on DVE (2x mode)
            o_t = pp.tile([128, NT, w], BF16, tag=f"tr{i % 2}_{lvl}",
                          name=f"{r}tr{d}_{lvl}")
            with nc.allow_low_precision(reason="bf16 tree partials"):
                nc.vector.tensor_add(o_t[:], t[:, :, 0:w], t[:, :, w:2 * w])
            t = o_t
            w //= 2
            lvl += 1
        g = T([128, NT, 2], BF16, f"g{d}")
        with nc.allow_low_precision(reason="bf16 G"):
            nc.vector.tensor_add(
                g[:], t[:, :, 0:1].to_broadcast([128, NT, 2]),
                t[:, :, 1:2].to_broadcast([128, NT, 2]))
        g2[d] = g
        si, sj = PAIRS[d]
        f = new_field(f"d{d}")
        if d in POOL_DS:
            t2 = sg.tile([128, NT, O], BF16, tag=f"{r}t2p{d}",
                         name=f"{r}t2p{d}")
            nc.gpsimd.tensor_mul(t2[:], dc[si][:, 0:NT, :],
                                 dc[sj][:, 0:NT, :])
            with nc.allow_low_precision(reason="bf16 field"):
                nc.gpsimd.tensor_mul(
                    f[:, NPADF:NPADF + NT, :], t2[:],
                    g2[d][:, :, 0:1].to_broadcast([128, NT, O]))
        else:
            t2 = pp.tile([128, NT, O], BF16, tag=f"t2{i % 2}",
                         name=f"{r}t2{d}")
            nc.vector.tensor_mul(t2[:], dc[si][:, 0:NT, :],
                                 dc[sj][:, 0:NT, :])
            nc.vector.tensor_mul(_as4(f[:, NPADF:NPADF + NT, :], O),
                                 _as4(t2[:], O), _rep2(g2[d], O))
        if i == 1:
            fdiag = new_field("diag")
            nc.scalar.activation(fdiag[:, NPADF:NPADF + NT, :],
                                 d16[:, 0:NT, :], AF.Square)
            fs1 = new_field("s1")
            nc.vector.tensor_mul(_as4(fs1[:, NPADF:NPADF + NT, :], O),
                                 _as4(d16[:, 0:NT, :], O), _rep2(sx2, O))

    # ---- pass-major banded box matmuls (all 4 chunks' PSUM live)
    osb = T([128, NT, O], F32, "osb")
    with (
        tc.tile_pool(name=r + "psQ", bufs=1, space="PSUM") as psQ,
        tc.tile_pool(name=r + "psS", bufs=1, space="PSUM") as psS,
    ):
        ps_q = [psQ.tile([128, cw, O], F32, tag=f"{r}psq{ci}",
                         name=f"{r}psq{ci}") for ci, (c0, cw) in
                enumerate(CHUNKS)]
        ps_s = [psS.tile([128, cw, O], F32, tag=f"{r}pss{ci}",
                         name=f"{r}pss{ci}") for ci, (c0, cw) in
                enumerate(CHUNKS)]
        wq = [0] * len(CHUNKS)
        ws = [0] * len(CHUNKS)
        nwq = sum(len(PASS_SIDES[pi]) for pi, p in enumerate(PASSES)
                  if p[4] == "q")
        nws = sum(len(PASS_SIDES[pi]) for pi, p in enumerate(PASSES)
                  if p[4] == "s")

        def box_half(chunk_ids):
            """Pass-major over a half of the chunks: PE consumes each field
            as soon as it is built; the other half's phase D overlaps."""
            for pi, (pname, dkey, coeff, box, tgt_kind) in enumerate(PASSES):
                fkey = "diag" if pname == "diag" else (
                    "s1" if pname == "s1" else f"d{dkey}")
                f = fields[fkey]
                for (j, bi) in PASS_SIDES[pi]:
                    for ci in chunk_ids:
                        c0, cw = CHUNKS[ci]
                        if tgt_kind == "q":
                            tgt, first, last = ps_q[ci], wq[ci] == 0, \
                                wq[ci] == nwq - 1
                            wq[ci] += 1
                        else:
                            tgt, first, last = ps_s[ci], ws[ci] == 0, \
                                ws[ci] == nws - 1
                            ws[ci] += 1
                        nc.tensor.matmul(
                            tgt[:], bands_sb[:, bi, :],
                            f[:, NPADF + c0 + j:NPADF + c0 + j + cw, :],
                            start=first, stop=last, skip_group_check=True)

        # ---- normalize/emit per chunk: rr = (-Q)^-1/2 via ln+exp
        def phase_d(ci):
            c0, cw = CHUNKS[ci]
            lnq = pp.tile([128, cw, O], F32, tag=f"lnq{ci % 2}",
                          name=f"{r}lnq{ci}")
            nc.scalar.activation(lnq[:], ps_q[ci][:], AF.Ln, scale=-1.0,
                                 bias=clnb[:])
            rr = pp.tile([128, cw, O], F32, tag=f"rr{ci % 2}",
                         name=f"{r}rr{ci}")
            nc.scalar.activation(rr[:], lnq[:], AF.Exp, scale=-0.5)
            nc.vector.scalar_tensor_tensor(
                out=osb[:, c0:c0 + cw, :], in0=ps_s[ci][:],
                scalar=1.0 / 63.0, in1=rr[:], op0=OP.mult, op1=OP.mult)
            s2 = pp.tile([128, cw, O - 1], F32, tag=f"s2{ci % 2}",
                         name=f"{r}s2{ci}")
            nc.scalar.activation(s2[:], osb[:, c0:c0 + cw, 1:O], AF.Square)
            red = pp.tile([128, cw], F32, tag=f"red{ci % 2}",
                          name=f"{r}red{ci}")
            nc.vector.tensor_reduce(red[:], s2[:], axis=mybir.AxisListType.X,
                                    op=OP.add)
            ln0 = pp.tile([128, cw], F32, tag=f"ln0{ci % 2}",
                          name=f"{r}ln0{ci}")
            nc.scalar.activation(ln0[:], red[:], AF.Ln, bias=1.0)
            nc.scalar.activation(osb[:, c0:c0 + cw, 0], ln0[:], AF.Exp,
                                 scale=0.5)
            oview = out_ext[128 * c0:128 * (c0 + cw), :].rearrange(
                "(t p) c -> p t c", p=128)
            # cols 1..63 are final after the stt; ship them while the out0
            # subchain (s2/red/ln0/exp0) still runs, then the tiny col-0 DMA
            eng = nc.sync if ci % 2 == 0 else nc.scalar
            eng.dma_start(out=oview[:, :, 1:O], in_=osb[:, c0:c0 + cw, 1:O])
            eng.dma_start(out=oview[:, :, 0:1], in_=osb[:, c0:c0 + cw, 0:1])

        box_half([0, 1])
        phase_d(0)
        box_half([2])
        phase_d(1)
        box_half([3])
        phase_d(2)
        phase_d(3)


_NC_CACHE = None


def _get_nc():
    global _NC_CACHE
    if _NC_CACHE is None:
        _NC_CACHE = build_nc()
    return _NC_CACHE


def host_consts(kernels):
    # u = -l_inner(x,k) = x0*k0 - sum_{c>=1} x_c*k_c ; col O is sum_{c>=1} x_c
    gk_ext = np.zeros((C, O + 1), dtype=np.float32)
    gk_ext[:, :O] = kernels.astype(np.float32).T
    gk_ext[1:, :O] *= -1.0
    gk_ext[1:, O] = 1.0
    return gk_ext


def pad_image(img):
    """[56,56,64] -> host-padded [NT1*128, 64] on the 58x58 grid."""
    xp = np.zeros((NT1 * 128, C), dtype=np.float32)
    grid = xp[:GW * GW].reshape(GW, GW, C)
    grid[1:57, 1:57] = img
    return xp


def unpad_out(o):
    """[NP,64] padded field -> [56,56,64] interior."""
    return o[:GW * GW].reshape(GW, GW, O)[1:57, 1:57]


def core_inputs(x, kernels, core=0):
    import ml_dtypes
    xp = pad_image(np.asarray(x[core], dtype=np.float32))
    x16 = xp.astype(ml_dtypes.bfloat16)
    return {
        "x": xp,
        "x16": x16,
        "gk_ext": np.ascontiguousarray(host_consts(kernels)),
        "bands": np.ascontiguousarray(
            BANDS.transpose(1, 0, 2).astype(ml_dtypes.bfloat16)),
        "ident": np.eye(128, dtype=np.float32),
    }


def kernel(x, kernels):
    x = np.asarray(x, dtype=np.float32)
    kernels = np.asarray(kernels, dtype=np.float32)
    B = x.shape[0]
    assert x.shape == (B, H, W, C) and B == 8, x.shape
    nc = _get_nc()
    in_maps = [core_inputs(x, kernels, core=i) for i in range(8)]
    res = run_bass_kernel_spmd(nc, in_maps, core_ids=list(range(8)),
                               trace=bool(int(os.environ.get("KTRACE", "0"))))
    if res.exec_time_ns is not None:
        print(f"HW exec time: {res.exec_time_ns} ns")
    out = np.stack([unpad_out(res.results[i]["out"]) for i in range(8)])
    return out.astype(np.float32)


# revision 13
# speedup vs baseline: 1.0942x; 1.0379x over previous
"""LorentzConv2d Trainium2 kernel v4.

Full-input contract: kernel(x=[8,56,56,64], kernels=[64,64]) -> [8,56,56,64].
Data-parallel over batch: one image per NeuronCore (8 cores).

Per-core algorithm on the zero-padded 58x58 grid, linearized l = 58*gh+gw,
tiled l = 128*t + p (p = partition):
  u[l,o]   = sum_c x[l,c] g_c k[o,c]    (PE, f32; col O accumulates sx)
  D[l,o]   = acosh(u)^2 = ln(u + sqrt(u^2-1+g))^2   (ACT chain, bf16,
             pipelined per tile-group so D tiles emerge early)
  Q[l,o]   = -box3x3(D^2) + 2 sum_d box_d( D_si * D_sj * G_d )   (PE bands)
  S1[l,o]  = box3x3(sx*D)
  out_o    = (S1/63) * exp(-0.5 ln(-Q)) ; out_0 = exp(0.5 ln(1+sum out_o^2))
Shifted fields come from SBUF->SBUF partition-shift DMA copies spread over
the SP and ACT DGE queues (consts ride Pool's SWDGE to dodge the serialized
HWDGE).  A difference basis S={0,1,2,58,60,116,118} expresses every
window-pair offset d as sj-si, so only 13 shifted copies are needed.
Engine balance: DVE keeps the G products/trees and most pair muls; Pool
takes the vv add and the last two deltas' pair muls; ACT absorbs gx16,
x16 tail, and the diag field (Square).
"""

import os
import numpy as np

import concourse.bass as bass
import concourse.bacc as bacc
import concourse.tile as tile
from concourse import mybir
from concourse.bass_utils import run_bass_kernel_spmd

# Restrict activation tables to the two actually used; keeps ln+exp+square in
# one set so the scheduler emits exactly two LoadActFuncSets.
import concourse.bacc as _bacc_mod
from concourse.hw_specs import get_activation_tables as _orig_gat


def _gat(arch):
    # Keep every table at its original index (act_func_set_id is positional
    # in act_info.json), but empty the ones we don't want chosen.
    tabs = _orig_gat(arch)
    keep = {"sqrt_and_others", "natural_log_exp_and_others"}
    if keep <= set(tabs):
        return {k: (v if k in keep else set()) for k, v in tabs.items()}
    return tabs


_bacc_mod.get_activation_tables = _gat

F32 = mybir.dt.float32
BF16 = mybir.dt.bfloat16
AF = mybir.ActivationFunctionType
OP = mybir.AluOpType

# geometry
H = W = 56
C = 64
O = 64
GW = 58                  # padded grid width (58x58)
NT = 27                  # 128-row tiles covering 58*58=3364 (+ tail)
NP = NT * 128            # 3456
NT1 = NT + 1             # +1 zero tail tile for shifted reads
NPADF = 2                # leading zero pad tiles in f fields (sides j>=-2)
NTF = NT + 3             # f field tiles: 2 lead + NT + 1 trail
SQ_GUARD = 1e-4          # replaces the max(u, 1+eps) clamp inside sqrt

# (dh, dw) per positive window-pair offset d = 58*dh + dw
DELTAS = {1: (0, 1), 2: (0, 2), 56: (1, -2), 57: (1, -1), 58: (1, 0),
          59: (1, 1), 60: (1, 2), 114: (2, -2), 115: (2, -1), 116: (2, 0),
          117: (2, 1), 118: (2, 2)}
# d -> (si, sj) with d = sj - si, both in the copy basis
PAIRS = {1: (1, 2), 2: (0, 2), 56: (2, 58), 57: (1, 58), 58: (0, 58),
         59: (1, 60), 60: (0, 60), 114: (2, 116), 115: (1, 116),
         116: (0, 116), 117: (1, 118), 118: (0, 118)}
XSHIFTS = [2, 118, 58, 60, 116]   # unsigned x copies (xc_s), arrival order
GXSHIFTS = [1, 2]                 # signed (col0-negated) copies (gxc_s)
DSHIFTS = [1, 2, 118, 58, 60, 116]
# delta processing order: Pool-assigned deltas early so their G trees (DVE)
# and pair muls (Pool) clear the critical path; the rest by copy arrival
DORDER = [2, 1, 118, 117, 58, 57, 56, 60, 59, 116, 115, 114]
POOL_DS = {118, 117}     # deltas whose pair muls run on the Pool engine
# pass order for the PE box matmuls: Pool-computed fields last (they are
# ready early anyway), DVE fields in completion order
PORDER = [2, 1, 58, 57, 56, 60, 59, 116, 115, 114, 118, 117]
# dist-chain tile groups (pipelined): psu PSUM tiles are per-group
UGROUPS = [(0, 7), (7, 7), (14, 7), (21, 6)]


def _interval(d):
    return range(max(-1, -1 - d), min(1, 1 - d) + 1)


def _build_passes():
    """pass = (name, dkey_or_None, coeff, box_offsets(after -si), target).
    Ordered to match field completion order (PE consumes passes in order):
    the first two deltas' fields finish before diag/s1 are emitted."""
    box33 = [58 * a + b for a in (-1, 0, 1) for b in (-1, 0, 1)]

    def dpass(d):
        dh, dw = DELTAS[d]
        si, _ = PAIRS[d]
        box = [58 * a + b - si for a in _interval(dh) for b in _interval(dw)]
        return (f"d{d}", d, 2.0, box, "q")

    passes = [dpass(PORDER[0]), dpass(PORDER[1]),
              ("diag", None, -1.0, box33, "q"),
              ("s1", None, 1.0, box33, "s")]
    for d in PORDER[2:]:
        passes.append(dpass(d))
    return passes


def _build_bands(passes):
    """Banded-Toeplitz matrices. T[m, i] = coeff iff the source row m of tile
    c+j supplies out row i:  m = i + t - 128j for t in box."""
    mats = []
    sides = []
    for (_, _, coeff, box, _) in passes:
        plist = []
        for j in (-2, -1, 0, 1):
            T = np.zeros((128, 128), dtype=np.float32)
            for t in set(box):
                dd = t - 128 * j
                if -127 <= dd <= 127:
                    idx = np.arange(max(0, dd), 128 + min(0, dd))
                    T[idx, idx - dd] = coeff
            if np.any(T):
                plist.append((j, len(mats)))
                mats.append(T)
        sides.append(plist)
    return np.stack(mats), sides


PASSES = _build_passes()
BANDS, PASS_SIDES = _build_bands(PASSES)
NB = BANDS.shape[0]
CHUNKS = [(0, 8), (8, 8), (16, 8), (24, 3)]


def _shift_copy(nc, dst, src, s, eng_a=None, eng_b=None, groups=None):
    """dst[p, 0:NT, :] = src rows l+s (l = 128t+p), via two partition-shifted
    SBUF->SBUF DMAs. src is [128, NT1, inner] with a zero tail tile.
    groups: optional list of (t0, tn) to emit per-piece (earlier starts)."""
    eng_a = eng_a or nc.sync
    eng_b = eng_b or eng_a
    assert 0 < s < 128
    if groups is None:
        groups = [(0, NT)]
    for (t0, tn) in groups:
        eng_a.dma_start(out=dst[0:128 - s, t0:t0 + tn, :],
                        in_=src[s:128, t0:t0 + tn, :])
        eng_b.dma_start(out=dst[128 - s:128, t0:t0 + tn, :],
                        in_=src[0:s, t0 + 1:t0 + tn + 1, :])


def _rep2(t, n_inner):
    """[128, NT, 2] tile viewed as [128, NT, n_inner/2, 2] via paired
    stride-1 reads (keeps the DVE 16-bit 2x mode on broadcast multiplies)."""
    return t[:].unsqueeze(2).to_broadcast([128, NT, n_inner // 2, 2])


def _as4(ap, n_inner):
    """[128, NT, n_inner] AP viewed as [128, NT, n_inner/2, 2]."""
    return ap.rearrange("p t (a b) -> p t a b", b=2)


def build_nc(reps=1):
    nc = bacc.Bacc(None)
    # x arrives host-padded onto the 58x58 grid (+ zero tail tile)
    x_in = nc.declare_dram_parameter("x", [NT1 * 128, C], F32, isOutput=False)
    x16_in = nc.declare_dram_parameter("x16", [NT1 * 128, C], BF16,
                                       isOutput=False)
    gk_in = nc.declare_dram_parameter("gk_ext", [C, O + 1], F32,
                                      isOutput=False)
    # bands stored partition-major on host: [p][band][m]
    bands_in = nc.declare_dram_parameter("bands", [128, NB, 128], BF16,
                                         isOutput=False)
    id_in = nc.declare_dram_parameter("ident", [128, 128], F32, isOutput=False)
    # full padded output field; host extracts the 56x56 interior
    out_ext = nc.declare_dram_parameter("out", [NP, O], F32, isOutput=True)

    with tile.TileContext(nc) as tc:
        for rep in range(reps):
            with (
                tc.tile_pool(name=f"sg{rep}", bufs=1) as sg,
                tc.tile_pool(name=f"pp{rep}", bufs=1) as pp,
            ):
                _one_rep(nc, tc, sg, pp, x_in, x16_in, gk_in,
                         bands_in, id_in, out_ext, rep)
    nc.finalize()
    return nc


def _one_rep(nc, tc, sg, pp, x_in, x16_in, gk_in, bands_in, id_in,
             out_ext, rep):
    r = f"r{rep}_"

    def T(shape, dt, name):
        return sg.tile(shape, dt, tag=r + name, name=r + name)

    # ---- constants ride Pool's SWDGE: keeps the serialized HWDGE free for
    # the x load and the shift-copy stream.  id first (gates transposes).
    id_sb = T([128, 128], F32, "id")
    nc.gpsimd.dma_start(out=id_sb[:], in_=id_in[:])
    gk_sb = T([C, O + 1], F32, "gk")
    nc.gpsimd.dma_start(out=gk_sb[:], in_=gk_in[:])

    # ---- x f32 chunks on SP; chunk 0 first (gates transposes), then the
    # bf16 copy (feeds shift copies), then the remaining chunks
    x_sb = T([128, NT1, C], F32, "x_sb")
    xview = x_in.rearrange("(t p) c -> p t c", p=128)
    x16 = T([128, NT1, C], BF16, "x16")

    def _xchunk(gi):
        t0, tn = UGROUPS[gi]
        tn += 1 if t0 + tn == NT else 0
        nc.sync.dma_start(out=x_sb[:, t0:t0 + tn, :],
                          in_=xview[:, t0:t0 + tn, :])

    _xchunk(0)
    nc.sync.dma_start(out=x16[:],
                      in_=x16_in.rearrange("(t p) c -> p t c", p=128))
    _xchunk(1)
    _xchunk(2)
    _xchunk(3)

    # gx16 (col0-negated x16) on ACT: idle early, frees DVE
    gx16 = T([128, NT1, C], BF16, "gx16")
    nc.scalar.activation(gx16[:, :, 1:C], x16[:, :, 1:C], AF.Copy)
    nc.scalar.activation(gx16[:, :, 0:1], x16[:, :, 0:1], AF.Copy, scale=-1.0)

    # ---- x/gx shift copies: xc[2] first (feeds d=2 with gxc[0]=gx16).
    # All on the SP queue: it is idle after the x load, while the ACT queue
    # would serialize these behind the latency-critical dist chain.
    xc = {0: x16}
    gxc = {0: gx16}
    shift_jobs = ([("x", 2), ("gx", 1), ("gx", 2)]
                  + [("x", s) for s in XSHIFTS if s != 2])
    for i, (kind, s) in enumerate(shift_jobs):
        src, dstmap = (x16, xc) if kind == "x" else (gx16, gxc)
        dstmap[s] = T([128, NT, C], BF16, f"{kind}c{s}")
        _shift_copy(nc, dstmap[s], src, s, eng_a=nc.sync, eng_b=nc.sync)

    bands_sb = T([128, NB, 128], BF16, "bands")
    nc.gpsimd.dma_start(out=bands_sb[:], in_=bands_in[:])

    # ---- phase A: transposes + u matmuls + per-group dist chain.
    # Chain per tile-group so D tiles emerge early and the D-shift DMAs /
    # pair muls overlap the later groups' chain.
    d16 = T([128, NT1, O], BF16, "d16")
    nc.vector.memset(d16[:, NT, :], 0.0)
    sx_sb = T([128, NT], F32, "sx")
    cm1g = T([128, 1], F32, "cm1g")
    nc.gpsimd.memset(cm1g[:], -1.0 + SQ_GUARD)
    cmone = T([128, 1], F32, "cmone")
    nc.gpsimd.memset(cmone[:], -1.0)
    clnb = T([128, 1], F32, "clnb")
    nc.gpsimd.memset(clnb[:], 1e-30)
    sx2 = T([128, NT, 2], BF16, "sx2")

    with (
        tc.tile_pool(name=r + "psA", bufs=1, space="PSUM") as psA,
        tc.tile_pool(name=r + "psT", bufs=3, space="PSUM") as psT,
    ):
        xT = T([64, NT, 128], F32, "xT")
        psu_g = [psA.tile([128, 7, O + 1], F32, tag=f"{r}psu{i}",
                          name=f"{r}psu{i}") for i in range(4)]
        for gi, (t0, tn) in enumerate(UGROUPS):
            for i in range(tn):
                tl = t0 + i
                xt_ps = psT.tile([C, 128], F32)
                nc.tensor.transpose(xt_ps[:], x_sb[:, tl, :], id_sb[:])
                nc.scalar.copy(xT[:, tl, :], xt_ps[:])
                nc.tensor.matmul(psu_g[gi][:, i, :], xT[:, tl, :], gk_sb[:],
                                 start=True, stop=True)

        # per-group dist chain, ALL in the ln/exp act-table set (sqrt is
        # exp(0.5*ln(.)) so no table reloads ever happen):
        #   sq = u^2 ; rl1 = relu(sq-1+g) ; rt = exp(0.5*ln(rl1+eps))
        #   vv = u + rt   (Pool add; u staged to SBUF since Pool can't PSUM)
        #   D  = ln(1 + relu(vv-1))^2  (relu zeroes the all-zero pad rows)
        for gi, (t0, tn) in enumerate(UGROUPS):
            b = f"g{gi % 2}_"
            u_ps = psu_g[gi][:, :tn, 0:O]
            sq = pp.tile([128, 7, O], F32, tag=b + "sq", name=f"{r}sq{gi}")
            nc.scalar.activation(sq[:, :tn, :], u_ps, AF.Square)
            u_sb = pp.tile([128, 7, O], F32, tag=b + "u", name=f"{r}u{gi}")
            nc.scalar.copy(u_sb[:, :tn, :], u_ps)
            nc.scalar.copy(sx_sb[:, t0:t0 + tn], psu_g[gi][:, :tn, O])
            rl1 = pp.tile([128, 7, O], F32, tag=b + "rl1", name=f"{r}rl1{gi}")
            nc.scalar.activation(rl1[:, :tn, :], sq[:, :tn, :], AF.Relu,
                                 bias=cm1g[:])
            lnr = pp.tile([128, 7, O], F32, tag=b + "sq", name=f"{r}lnr{gi}")
            nc.scalar.activation(lnr[:, :tn, :], rl1[:, :tn, :], AF.Ln,
                                 bias=clnb[:])
            rt = pp.tile([128, 7, O], F32, tag=b + "rl1", name=f"{r}rt{gi}")
            nc.scalar.activation(rt[:, :tn, :], lnr[:, :tn, :], AF.Exp,
                                 scale=0.5)
            vv = pp.tile([128, 7, O], F32, tag=b + "sq", name=f"{r}vv{gi}")
            nc.gpsimd.tensor_add(vv[:, :tn, :], u_sb[:, :tn, :],
                                 rt[:, :tn, :])
            rl = pp.tile([128, 7, O], F32, tag=b + "rl1", name=f"{r}rl{gi}")
            nc.scalar.activation(rl[:, :tn, :], vv[:, :tn, :], AF.Relu,
                                 bias=cmone[:])
            lnv = pp.tile([128, 7, O], F32, tag=b + "sq", name=f"{r}lnv{gi}")
            nc.scalar.activation(lnv[:, :tn, :], rl[:, :tn, :], AF.Ln,
                                 bias=1.0)
            nc.scalar.activation(d16[:, t0:t0 + tn, :], lnv[:, :tn, :],
                                 AF.Square)
        nc.gpsimd.tensor_copy(sx2[:], sx_sb[:].unsqueeze(2).to_broadcast(
            [128, NT, 2]))

    # ---- shifted D copies: s=1,2 split in halves (earlier starts, feed the
    # first pair muls); long shifts full-tensor.  Spread SP/ACT queues.
    HALVES = [(0, 14), (14, 13)]
    dc = {0: d16}
    for i, s in enumerate(DSHIFTS):
        dc[s] = T([128, NT, O], BF16, f"dc{s}")
        groups = HALVES if s in (1, 2) else None
        _shift_copy(nc, dc[s], d16, s, eng_a=nc.sync, eng_b=nc.sync,
                    groups=groups)

    # ---- fields: per-delta pipeline with G-product lookahead.  DVE does the
    # G products/trees and most pair muls; Pool takes POOL_DS' pair muls;
    # ACT takes the diag field (Square of d16).
    fields = {}

    def new_field(key):
        f = sg.tile([128, NTF, O], BF16, tag=f"{r}f{key}", name=f"{r}f{key}")
        nc.gpsimd.memset(f[:, 0:NPADF, :], 0.0)
        nc.gpsimd.memset(f[:, NPADF + NT:, :], 0.0)
        fields[key] = f
        return f

    g2 = {}
    tg = {}
    LOOK = 3

    def emit_prod(i):
        d = DORDER[i]
        si, sj = PAIRS[d]
        t = pp.tile([128, NT, C], BF16, tag=f"tg{i % (LOOK + 1)}",
                    name=f"{r}tg{d}")
        nc.vector.tensor_mul(t[:], gxc[si][:, 0:NT, :], xc[sj][:, 0:NT, :])
        tg[d] = t

    for i in range(LOOK):
        emit_prod(i)
    for i, d in enumerate(DORDER):
        if i + LOOK < len(DORDER):
            emit_prod(i + LOOK)
        t = tg[d]
        w = C // 2
        lvl = 0
        while w >= 2:  # bf16 halving levels on DVE (2x mode)
            o_t = pp.tile([128, NT, w], BF16, tag=f"tr{i % 2}_{lvl}",
                          name=f"{r}tr{d}_{lvl}")
            with nc.allow_low_precision(reason="bf16 tree partials"):
                nc.vector.tensor_add(o_t[:], t[:, :, 0:w], t[:, :, w:2 * w])
            t = o_t
            w //= 2
            lvl += 1
        g = T([128, NT, 2], BF16, f"g{d}")
        with nc.allow_low_precision(reason="bf16 G"):
            nc.vector.tensor_add(
                g[:], t[:, :, 0:1].to_broadcast([128, NT, 2]),
                t[:, :, 1:2].to_broadcast([128, NT, 2]))
        g2[d] = g
        si, sj = PAIRS[d]
        f = new_field(f"d{d}")
        if d in POOL_DS:
            t2 = sg.tile([128, NT, O], BF16, tag=f"{r}t2p{d}",
                         name=f"{r}t2p{d}")
            nc.gpsimd.tensor_mul(t2[:], dc[si][:, 0:NT, :],
                                 dc[sj][:, 0:NT, :])
            with nc.allow_low_precision(reason="bf16 field"):
                nc.gpsimd.tensor_mul(
                    f[:, NPADF:NPADF + NT, :], t2[:],
                    g2[d][:, :, 0:1].to_broadcast([128, NT, O]))
        else:
            t2 = pp.tile([128, NT, O], BF16, tag=f"t2{i % 2}",
                         name=f"{r}t2{d}")
            nc.vector.tensor_mul(t2[:], dc[si][:, 0:NT, :],
                                 dc[sj][:, 0:NT, :])
            nc.vector.tensor_mul(_as4(f[:, NPADF:NPADF + NT, :], O),
                                 _as4(t2[:], O), _rep2(g2[d], O))
        if i == 1:
            fdiag = new_field("diag")
            nc.scalar.activation(fdiag[:, NPADF:NPADF + NT, :],
                                 d16[:, 0:NT, :], AF.Square)
            fs1 = new_field("s1")
            nc.vector.tensor_mul(_as4(fs1[:, NPADF:NPADF + NT, :], O),
                                 _as4(d16[:, 0:NT, :], O), _rep2(sx2, O))

    # ---- pass-major banded box matmuls (all 4 chunks' PSUM live)
    osb = T([128, NT, O], F32, "osb")
    with (
        tc.tile_pool(name=r + "psQ", bufs=1, space="PSUM") as psQ,
        tc.tile_pool(name=r + "psS", bufs=1, space="PSUM") as psS,
    ):
        ps_q = [psQ.tile([128, cw, O], F32, tag=f"{r}psq{ci}",
                         name=f"{r}psq{ci}") for ci, (c0, cw) in
                enumerate(CHUNKS)]
        ps_s = [psS.tile([128, cw, O], F32, tag=f"{r}pss{ci}",
                         name=f"{r}pss{ci}") for ci, (c0, cw) in
                enumerate(CHUNKS)]
        wq = [0] * len(CHUNKS)
        ws = [0] * len(CHUNKS)
        nwq = sum(len(PASS_SIDES[pi]) for pi, p in enumerate(PASSES)
                  if p[4] == "q")
        nws = sum(len(PASS_SIDES[pi]) for pi, p in enumerate(PASSES)
                  if p[4] == "s")

        def box_half(chunk_ids):
            """Pass-major over a half of the chunks: PE consumes each field
            as soon as it is built; the other half's phase D overlaps."""
            for pi, (pname, dkey, coeff, box, tgt_kind) in enumerate(PASSES):
                fkey = "diag" if pname == "diag" else (
                    "s1" if pname == "s1" else f"d{dkey}")
                f = fields[fkey]
                for (j, bi) in PASS_SIDES[pi]:
                    for ci in chunk_ids:
                        c0, cw = CHUNKS[ci]
                        if tgt_kind == "q":
                            tgt, first, last = ps_q[ci], wq[ci] == 0, \
                                wq[ci] == nwq - 1
                            wq[ci] += 1
                        else:
                            tgt, first, last = ps_s[ci], ws[ci] == 0, \
                                ws[ci] == nws - 1
                            ws[ci] += 1
                        nc.tensor.matmul(
                            tgt[:], bands_sb[:, bi, :],
                            f[:, NPADF + c0 + j:NPADF + c0 + j + cw, :],
                            start=first, stop=last, skip_group_check=True)

        # ---- normalize/emit per chunk: rr = (-Q)^-1/2 via ln+exp
        def phase_d(ci):
            c0, cw = CHUNKS[ci]
            lnq = pp.tile([128, cw, O], F32, tag=f"lnq{ci % 2}",
                          name=f"{r}lnq{ci}")
            nc.scalar.activation(lnq[:], ps_q[ci][:], AF.Ln, scale=-1.0,
                                 bias=clnb[:])
            rr = pp.tile([128, cw, O], F32, tag=f"rr{ci % 2}",
                         name=f"{r}rr{ci}")
            nc.scalar.activation(rr[:], lnq[:], AF.Exp, scale=-0.5)
            nc.vector.scalar_tensor_tensor(
                out=osb[:, c0:c0 + cw, :], in0=ps_s[ci][:],
                scalar=1.0 / 63.0, in1=rr[:], op0=OP.mult, op1=OP.mult)
            s2 = pp.tile([128, cw, O - 1], F32, tag=f"s2{ci % 2}",
                         name=f"{r}s2{ci}")
            nc.scalar.activation(s2[:], osb[:, c0:c0 + cw, 1:O], AF.Square)
            red = pp.tile([128, cw], F32, tag=f"red{ci % 2}",
                          name=f"{r}red{ci}")
            nc.vector.tensor_reduce(red[:], s2[:], axis=mybir.AxisListType.X,
                                    op=OP.add)
            ln0 = pp.tile([128, cw], F32, tag=f"ln0{ci % 2}",
                          name=f"{r}ln0{ci}")
            nc.scalar.activation(ln0[:], red[:], AF.Ln, bias=1.0)
            nc.scalar.activation(osb[:, c0:c0 + cw, 0], ln0[:], AF.Exp,
                                 scale=0.5)
            oview = out_ext[128 * c0:128 * (c0 + cw), :].rearrange(
                "(t p) c -> p t c", p=128)
            # cols 1..63 are final after the stt; ship them while the out0
            # subchain (s2/red/ln0/exp0) still runs, then the tiny col-0 DMA
            eng = nc.sync if ci % 2 == 0 else nc.scalar
            eng.dma_start(out=oview[:, :, 1:O], in_=osb[:, c0:c0 + cw, 1:O])
            eng.dma_start(out=oview[:, :, 0:1], in_=osb[:, c0:c0 + cw, 0:1])

        # all chunks pass-major: fields are consumed as they complete, and
        # the four phase-D chains pipeline on ACT/DVE right at the end
        box_half([0, 1, 2, 3])
        phase_d(0)
        phase_d(1)
        phase_d(2)
        phase_d(3)


_NC_CACHE = None


def _get_nc():
    global _NC_CACHE
    if _NC_CACHE is None:
        _NC_CACHE = build_nc()
    return _NC_CACHE


def host_consts(kernels):
    # u = -l_inner(x,k) = x0*k0 - sum_{c>=1} x_c*k_c ; col O is sum_{c>=1} x_c
    gk_ext = np.zeros((C, O + 1), dtype=np.float32)
    gk_ext[:, :O] = kernels.astype(np.float32).T
    gk_ext[1:, :O] *= -1.0
    gk_ext[1:, O] = 1.0
    return gk_ext


def pad_image(img):
    """[56,56,64] -> host-padded [NT1*128, 64] on the 58x58 grid."""
    xp = np.zeros((NT1 * 128, C), dtype=np.float32)
    grid = xp[:GW * GW].reshape(GW, GW, C)
    grid[1:57, 1:57] = img
    return xp


def unpad_out(o):
    """[NP,64] padded field -> [56,56,64] interior."""
    return o[:GW * GW].reshape(GW, GW, O)[1:57, 1:57]


def core_inputs(x, kernels, core=0):
    import ml_dtypes
    xp = pad_image(np.asarray(x[core], dtype=np.float32))
    x16 = xp.astype(ml_dtypes.bfloat16)
    return {
        "x": xp,
        "x16": x16,
        "gk_ext": np.ascontiguousarray(host_consts(kernels)),
        "bands": np.ascontiguousarray(
            BANDS.transpose(1, 0, 2).astype(ml_dtypes.bfloat16)),
        "ident": np.eye(128, dtype=np.float32),
    }


def kernel(x, kernels):
    x = np.asarray(x, dtype=np.float32)
    kernels = np.asarray(kernels, dtype=np.float32)
    B = x.shape[0]
    assert x.shape == (B, H, W, C) and B == 8, x.shape
    nc = _get_nc()
    in_maps = [core_inputs(x, kernels, core=i) for i in range(8)]
    res = run_bass_kernel_spmd(nc, in_maps, core_ids=list(range(8)),
                               trace=bool(int(os.environ.get("KTRACE", "0"))))
    if res.exec_time_ns is not None:
        print(f"HW exec time: {res.exec_time_ns} ns")
    out = np.stack([unpad_out(res.results[i]["out"]) for i in range(8)])
    return out.astype(np.float32)


# revision 14
# speedup vs baseline: 1.1138x; 1.0179x over previous
"""LorentzConv2d Trainium2 kernel v5.

Full-input contract: kernel(x=[8,56,56,64], kernels=[64,64]) -> [8,56,56,64].
Data-parallel over batch: one image per NeuronCore (8 cores).

Per-core algorithm on the zero-padded 58x58 grid, linearized l = 58*gh+gw,
tiled l = 128*t + p (p = partition):
  u[l,o]   = sum_c x[l,c] g_c k[o,c]    (PE, f32; col O accumulates sx)
  D[l,o]   = acosh(u)^2 = ln(u + sqrt(u^2-1+g))^2   (ACT chain, bf16,
             per-group pipelined; sqrt via exp(0.5 ln) so the whole kernel
             stays in the single ln/exp act-table set)
  Q[l,o]   = -box3x3(D^2) + 2 sum_d box_d( D_si * D_sj * G_d )   (PE bands)
  S1[l,o]  = box3x3(sx*D)
  out_o    = (S1/63) * exp(-0.5 ln(-Q)) ; out_0 = exp(0.5 ln(1+sum out_o^2))
Shifted fields come from SBUF->SBUF partition-shift DMA copies, all on the
SP queue (consts ride Pool's SWDGE).  A difference basis
S={0,1,2,58,60,116,118} expresses every window-pair offset d as sj-si, so
only 13 shifted copies are needed.  Edge box matmuls are range-clamped so
fields carry no zero-pad tiles (no memsets, smaller SBUF).
Engine balance: DVE does x16/gx16 derivation, G products/trees and 8
deltas' pair muls; Pool takes the vv add and 4 deltas' pair muls; ACT runs
the dist chain, half the xT copies and the diag field.
"""

import os
import numpy as np

import concourse.bass as bass
import concourse.bacc as bacc
import concourse.tile as tile
from concourse import mybir
from concourse.bass_utils import run_bass_kernel_spmd

import concourse.bacc as _bacc_mod
from concourse.hw_specs import get_activation_tables as _orig_gat


def _gat(arch):
    tabs = _orig_gat(arch)
    keep = {"sqrt_and_others", "natural_log_exp_and_others"}
    if keep <= set(tabs):
        return {k: (v if k in keep else set()) for k, v in tabs.items()}
    return tabs


_bacc_mod.get_activation_tables = _gat

F32 = mybir.dt.float32
BF16 = mybir.dt.bfloat16
AF = mybir.ActivationFunctionType
OP = mybir.AluOpType

# geometry
H = W = 56
C = 64
O = 64
GW = 58                  # padded grid width (58x58)
NT = 27                  # 128-row tiles covering 58*58=3364 (+ tail)
NP = NT * 128            # 3456
NT1 = NT + 1             # +1 zero tail tile for shifted reads
SQ_GUARD = 1e-4          # replaces the max(u, 1+eps) clamp inside sqrt

# (dh, dw) per positive window-pair offset d = 58*dh + dw
DELTAS = {1: (0, 1), 2: (0, 2), 56: (1, -2), 57: (1, -1), 58: (1, 0),
          59: (1, 1), 60: (1, 2), 114: (2, -2), 115: (2, -1), 116: (2, 0),
          117: (2, 1), 118: (2, 2)}
# d -> (si, sj) with d = sj - si, both in the copy basis
PAIRS = {1: (1, 2), 2: (0, 2), 56: (2, 58), 57: (1, 58), 58: (0, 58),
         59: (1, 60), 60: (0, 60), 114: (2, 116), 115: (1, 116),
         116: (0, 116), 117: (1, 118), 118: (0, 118)}
XSHIFTS = [2, 118, 116, 58, 60]   # unsigned x copies (xc_s), arrival order
GXSHIFTS = [1, 2]                 # signed (col0-negated) copies (gxc_s)
DSHIFTS = [1, 2, 118, 116, 58, 60]
# delta processing order: Pool-assigned deltas early so their G trees (DVE)
# clear before Pool needs them; the rest by shift-copy arrival
DORDER = [2, 1, 118, 117, 115, 114, 58, 57, 56, 60, 59, 116]
POOL_DS = {118, 117, 115, 114}   # deltas whose pair muls run on Pool
# pass order for the PE box matmuls: Pool fields last (ready early anyway)
PORDER = [2, 1, 58, 57, 56, 60, 59, 116, 118, 117, 115, 114]
# dist-chain tile groups (pipelined): psu PSUM tiles are per-group
UGROUPS = [(0, 7), (7, 7), (14, 7), (21, 6)]


def _interval(d):
    return range(max(-1, -1 - d), min(1, 1 - d) + 1)


def _build_passes():
    box33 = [58 * a + b for a in (-1, 0, 1) for b in (-1, 0, 1)]

    def dpass(d):
        dh, dw = DELTAS[d]
        si, _ = PAIRS[d]
        box = [58 * a + b - si for a in _interval(dh) for b in _interval(dw)]
        return (f"d{d}", d, 2.0, box, "q")

    passes = [dpass(PORDER[0]), dpass(PORDER[1]),
              ("diag", None, -1.0, box33, "q"),
              ("s1", None, 1.0, box33, "s")]
    for d in PORDER[2:]:
        passes.append(dpass(d))
    return passes


def _build_bands(passes):
    """Banded-Toeplitz matrices. T[m, i] = coeff iff the source row m of tile
    c+j supplies out row i:  m = i + t - 128j for t in box.  Side j=0 first
    so the first matmul of every chunk covers the full PSUM tile."""
    mats = []
    sides = []
    for (_, _, coeff, box, _) in passes:
        plist = []
        for j in (0, -1, 1):
            T = np.zeros((128, 128), dtype=np.float32)
            for t in set(box):
                dd = t - 128 * j
                if -127 <= dd <= 127:
                    idx = np.arange(max(0, dd), 128 + min(0, dd))
                    T[idx, idx - dd] = coeff
            if np.any(T):
                plist.append((j, len(mats)))
                mats.append(T)
        sides.append(plist)
    return np.stack(mats), sides


PASSES = _build_passes()
BANDS, PASS_SIDES = _build_bands(PASSES)
NB = BANDS.shape[0]
CHUNKS = [(0, 8), (8, 8), (16, 8), (24, 3)]


def _shift_copy(nc, dst, src, s, eng_a=None, eng_b=None, groups=None):
    """dst[p, 0:NT, :] = src rows l+s (l = 128t+p), via two partition-shifted
    SBUF->SBUF DMAs. src is [128, NT1, inner] with a zero tail tile."""
    eng_a = eng_a or nc.sync
    eng_b = eng_b or eng_a
    assert 0 < s < 128
    if groups is None:
        groups = [(0, NT)]
    for (t0, tn) in groups:
        eng_a.dma_start(out=dst[0:128 - s, t0:t0 + tn, :],
                        in_=src[s:128, t0:t0 + tn, :])
        eng_b.dma_start(out=dst[128 - s:128, t0:t0 + tn, :],
                        in_=src[0:s, t0 + 1:t0 + tn + 1, :])


def _rep2(t, n_inner):
    """[128, NT, 2] tile viewed as [128, NT, n_inner/2, 2] via paired
    stride-1 reads (keeps the DVE 16-bit 2x mode on broadcast multiplies)."""
    return t[:].unsqueeze(2).to_broadcast([128, NT, n_inner // 2, 2])


def _as4(ap, n_inner):
    """[128, NT, n_inner] AP viewed as [128, NT, n_inner/2, 2]."""
    return ap.rearrange("p t (a b) -> p t a b", b=2)


def build_nc(reps=1):
    nc = bacc.Bacc(None)
    # x arrives host-padded onto the 58x58 grid (+ zero tail tile)
    x_in = nc.declare_dram_parameter("x", [NT1 * 128, C], F32, isOutput=False)
    gk_in = nc.declare_dram_parameter("gk_ext", [C, O + 1], F32,
                                      isOutput=False)
    bands_in = nc.declare_dram_parameter("bands", [128, NB, 128], BF16,
                                         isOutput=False)
    id_in = nc.declare_dram_parameter("ident", [128, 128], F32, isOutput=False)
    out_ext = nc.declare_dram_parameter("out", [NP, O], F32, isOutput=True)

    with tile.TileContext(nc) as tc:
        for rep in range(reps):
            with (
                tc.tile_pool(name=f"sg{rep}", bufs=1) as sg,
                tc.tile_pool(name=f"pp{rep}", bufs=1) as pp,
            ):
                _one_rep(nc, tc, sg, pp, x_in, gk_in,
                         bands_in, id_in, out_ext, rep)
    nc.finalize()
    return nc


def _one_rep(nc, tc, sg, pp, x_in, gk_in, bands_in, id_in, out_ext, rep):
    r = f"r{rep}_"

    def T(shape, dt, name):
        return sg.tile(shape, dt, tag=r + name, name=r + name)

    # ---- consts on Pool's SWDGE (id first: gates transposes)
    id_sb = T([128, 128], F32, "id")
    nc.gpsimd.dma_start(out=id_sb[:], in_=id_in[:])
    gk_sb = T([C, O + 1], F32, "gk")
    nc.gpsimd.dma_start(out=gk_sb[:], in_=gk_in[:])

    # ---- x f32 chunks on SP (the only early HWDGE users)
    x_sb = T([128, NT1, C], F32, "x_sb")
    xview = x_in.rearrange("(t p) c -> p t c", p=128)

    def _xrange(gi):
        t0, tn = UGROUPS[gi]
        tn += 1 if t0 + tn == NT else 0
        return t0, tn

    for gi in range(4):
        t0, tn = _xrange(gi)
        nc.sync.dma_start(out=x_sb[:, t0:t0 + tn, :],
                          in_=xview[:, t0:t0 + tn, :])

    # ---- x16/gx16 derived on DVE per chunk (idle head; no HBM x16 load)
    x16 = T([128, NT1, C], BF16, "x16")
    gx16 = T([128, NT1, C], BF16, "gx16")
    for gi in range(4):
        t0, tn = _xrange(gi)
        nc.vector.tensor_copy(x16[:, t0:t0 + tn, :], x_sb[:, t0:t0 + tn, :])
        nc.vector.tensor_copy(gx16[:, t0:t0 + tn, :], x16[:, t0:t0 + tn, :])
    nc.vector.tensor_scalar_mul(gx16[:, :, 0], gx16[:, :, 0], -1.0)

    # ---- x/gx shift copies, all on the idle SP queue
    xc = {0: x16}
    gxc = {0: gx16}
    shift_jobs = ([("x", 2), ("gx", 1), ("gx", 2)]
                  + [("x", s) for s in XSHIFTS if s != 2])
    for (kind, s) in shift_jobs:
        src, dstmap = (x16, xc) if kind == "x" else (gx16, gxc)
        dstmap[s] = T([128, NT, C], BF16, f"{kind}c{s}")
        _shift_copy(nc, dstmap[s], src, s, eng_a=nc.sync, eng_b=nc.sync)

    # ---- phase A: transposes + u matmuls + per-group dist chain
    d16 = T([128, NT1, O], BF16, "d16")
    nc.vector.memset(d16[:, NT, :], 0.0)
    sx_sb = T([128, NT], F32, "sx")
    cm1g = T([128, 1], F32, "cm1g")
    nc.gpsimd.memset(cm1g[:], -1.0 + SQ_GUARD)
    cmone = T([128, 1], F32, "cmone")
    nc.gpsimd.memset(cmone[:], -1.0)
    clnb = T([128, 1], F32, "clnb")
    nc.gpsimd.memset(clnb[:], 1e-30)
    sx2 = T([128, NT, 2], BF16, "sx2")

    with (
        tc.tile_pool(name=r + "psA", bufs=1, space="PSUM") as psA,
        tc.tile_pool(name=r + "psT", bufs=3, space="PSUM") as psT,
    ):
        xT = T([64, NT, 128], F32, "xT")
        psu_g = [psA.tile([128, 7, O + 1], F32, tag=f"{r}psu{i}",
                          name=f"{r}psu{i}") for i in range(4)]
        for gi, (t0, tn) in enumerate(UGROUPS):
            for i in range(tn):
                tl = t0 + i
                xt_ps = psT.tile([C, 128], F32)
                nc.tensor.transpose(xt_ps[:], x_sb[:, tl, :], id_sb[:])
                # PSUM->SBUF xT copies: DVE for the first half (idle head),
                # ACT for the rest
                if gi < 2:
                    nc.vector.tensor_copy(xT[:, tl, :], xt_ps[:])
                else:
                    nc.scalar.copy(xT[:, tl, :], xt_ps[:])
                nc.tensor.matmul(psu_g[gi][:, i, :], xT[:, tl, :], gk_sb[:],
                                 start=True, stop=True)

        # per-group dist chain, all in the ln/exp act-table set:
        #   sq = u^2 ; rl1 = relu(sq-1+g) ; rt = exp(0.5*ln(rl1+eps))
        #   vv = u + rt   (Pool add; u staged to SBUF since Pool can't PSUM)
        #   D  = ln(1 + relu(vv-1))^2  (relu zeroes the all-zero pad rows)
        for gi, (t0, tn) in enumerate(UGROUPS):
            b = f"g{gi % 2}_"
            u_ps = psu_g[gi][:, :tn, 0:O]
            sq = pp.tile([128, 7, O], F32, tag=b + "sq", name=f"{r}sq{gi}")
            nc.scalar.activation(sq[:, :tn, :], u_ps, AF.Square)
            u_sb = pp.tile([128, 7, O], F32, tag=b + "u", name=f"{r}u{gi}")
            nc.scalar.copy(u_sb[:, :tn, :], u_ps)
            nc.scalar.copy(sx_sb[:, t0:t0 + tn], psu_g[gi][:, :tn, O])
            rl1 = pp.tile([128, 7, O], F32, tag=b + "rl1", name=f"{r}rl1{gi}")
            nc.scalar.activation(rl1[:, :tn, :], sq[:, :tn, :], AF.Relu,
                                 bias=cm1g[:])
            lnr = pp.tile([128, 7, O], F32, tag=b + "sq", name=f"{r}lnr{gi}")
            nc.scalar.activation(lnr[:, :tn, :], rl1[:, :tn, :], AF.Ln,
                                 bias=clnb[:])
            rt = pp.tile([128, 7, O], F32, tag=b + "rl1", name=f"{r}rt{gi}")
            nc.scalar.activation(rt[:, :tn, :], lnr[:, :tn, :], AF.Exp,
                                 scale=0.5)
            vv = pp.tile([128, 7, O], F32, tag=b + "sq", name=f"{r}vv{gi}")
            nc.gpsimd.tensor_add(vv[:, :tn, :], u_sb[:, :tn, :],
                                 rt[:, :tn, :])
            rl = pp.tile([128, 7, O], F32, tag=b + "rl1", name=f"{r}rl{gi}")
            nc.scalar.activation(rl[:, :tn, :], vv[:, :tn, :], AF.Relu,
                                 bias=cmone[:])
            lnv = pp.tile([128, 7, O], F32, tag=b + "sq", name=f"{r}lnv{gi}")
            nc.scalar.activation(lnv[:, :tn, :], rl[:, :tn, :], AF.Ln,
                                 bias=1.0)
            nc.scalar.activation(d16[:, t0:t0 + tn, :], lnv[:, :tn, :],
                                 AF.Square)
        nc.gpsimd.tensor_copy(sx2[:], sx_sb[:].unsqueeze(2).to_broadcast(
            [128, NT, 2]))

    # ---- shifted D copies: s=1,2 in halves (earlier starts); all on SP
    HALVES = [(0, 14), (14, 13)]
    dc = {0: d16}
    for s in DSHIFTS:
        dc[s] = T([128, NT, O], BF16, f"dc{s}")
        groups = HALVES if s in (1, 2) else None
        _shift_copy(nc, dc[s], d16, s, eng_a=nc.sync, eng_b=nc.sync,
                    groups=groups)

    # bands load late on SWDGE: needed only when the box matmuls start
    bands_sb = T([128, NB, 128], BF16, "bands")
    nc.gpsimd.dma_start(out=bands_sb[:], in_=bands_in[:])

    # ---- fields ([128, NT, O], no pads: edge matmuls are range-clamped)
    fields = {}

    def new_field(key):
        f = sg.tile([128, NT, O], BF16, tag=f"{r}f{key}", name=f"{r}f{key}")
        fields[key] = f
        return f

    g2 = {}
    tg = {}
    LOOK = 3

    def emit_prod(i):
        d = DORDER[i]
        si, sj = PAIRS[d]
        t = pp.tile([128, NT, C], BF16, tag=f"tg{i % (LOOK + 1)}",
                    name=f"{r}tg{d}")
        nc.vector.tensor_mul(t[:], gxc[si][:, 0:NT, :], xc[sj][:, 0:NT, :])
        tg[d] = t

    for i in range(LOOK):
        emit_prod(i)
    for i, d in enumerate(DORDER):
        if i + LOOK < len(DORDER):
            emit_prod(i + LOOK)
        t = tg[d]
        w = C // 2
        lvl = 0
        while w >= 2:  # bf16 halving levels on DVE (2x mode)
            o_t = pp.tile([128, NT, w], BF16, tag=f"tr{i % 2}_{lvl}",
                          name=f"{r}tr{d}_{lvl}")
            with nc.allow_low_precision(reason="bf16 tree partials"):
                nc.vector.tensor_add(o_t[:], t[:, :, 0:w], t[:, :, w:2 * w])
            t = o_t
            w //= 2
            lvl += 1
        g = T([128, NT, 2], BF16, f"g{d}")
        with nc.allow_low_precision(reason="bf16 G"):
            nc.vector.tensor_add(
                g[:], t[:, :, 0:1].to_broadcast([128, NT, 2]),
                t[:, :, 1:2].to_broadcast([128, NT, 2]))
        g2[d] = g
        si, sj = PAIRS[d]
        f = new_field(f"d{d}")
        if d in POOL_DS:
            t2 = sg.tile([128, NT, O], BF16, tag=f"{r}t2p{d}",
                         name=f"{r}t2p{d}")
            nc.gpsimd.tensor_mul(t2[:], dc[si][:, 0:NT, :],
                                 dc[sj][:, 0:NT, :])
            with nc.allow_low_precision(reason="bf16 field"):
                nc.gpsimd.tensor_mul(
                    f[:], t2[:],
                    g2[d][:, :, 0:1].to_broadcast([128, NT, O]))
        else:
            t2 = pp.tile([128, NT, O], BF16, tag=f"t2{i % 2}",
                         name=f"{r}t2{d}")
            nc.vector.tensor_mul(t2[:], dc[si][:, 0:NT, :],
                                 dc[sj][:, 0:NT, :])
            nc.vector.tensor_mul(_as4(f[:], O), _as4(t2[:], O),
                                 _rep2(g2[d], O))
        if i == 1:
            fdiag = new_field("diag")
            nc.scalar.activation(fdiag[:], d16[:, 0:NT, :], AF.Square)
            fs1 = new_field("s1")
            nc.vector.tensor_mul(_as4(fs1[:], O), _as4(d16[:, 0:NT, :], O),
                                 _rep2(sx2, O))

    # ---- pass-major banded box matmuls over all 4 chunks; edge clamped
    osb = T([128, NT, O], F32, "osb")
    with (
        tc.tile_pool(name=r + "psQ", bufs=1, space="PSUM") as psQ,
        tc.tile_pool(name=r + "psS", bufs=1, space="PSUM") as psS,
    ):
        ps_q = [psQ.tile([128, cw, O], F32, tag=f"{r}psq{ci}",
                         name=f"{r}psq{ci}") for ci, (c0, cw) in
                enumerate(CHUNKS)]
        ps_s = [psS.tile([128, cw, O], F32, tag=f"{r}pss{ci}",
                         name=f"{r}pss{ci}") for ci, (c0, cw) in
                enumerate(CHUNKS)]
        # per-chunk counters over CLAMPED emission counts
        def _emit_count(tgt_kind, ci):
            c0, cw = CHUNKS[ci]
            n = 0
            for pi, p in enumerate(PASSES):
                if p[4] != tgt_kind:
                    continue
                for (j, _) in PASS_SIDES[pi]:
                    if min(NT, c0 + j + cw) - max(0, c0 + j) > 0:
                        n += 1
            return n

        nq = [_emit_count("q", ci) for ci in range(4)]
        ns = [_emit_count("s", ci) for ci in range(4)]
        wq = [0] * len(CHUNKS)
        ws = [0] * len(CHUNKS)

        def box_all():
            for pi, (pname, dkey, coeff, box, tgt_kind) in enumerate(PASSES):
                fkey = "diag" if pname == "diag" else (
                    "s1" if pname == "s1" else f"d{dkey}")
                f = fields[fkey]
                for (j, bi) in PASS_SIDES[pi]:
                    for ci in range(4):
                        c0, cw = CHUNKS[ci]
                        s0 = max(0, c0 + j)
                        s1 = min(NT, c0 + j + cw)
                        if s1 <= s0:
                            continue
                        oo = s0 - (c0 + j)
                        if tgt_kind == "q":
                            tgt, first, last = ps_q[ci], wq[ci] == 0, \
                                wq[ci] == nq[ci] - 1
                            wq[ci] += 1
                        else:
                            tgt, first, last = ps_s[ci], ws[ci] == 0, \
                                ws[ci] == ns[ci] - 1
                            ws[ci] += 1
                        nc.tensor.matmul(
                            tgt[:, oo:oo + (s1 - s0), :],
                            bands_sb[:, bi, :], f[:, s0:s1, :],
                            start=first, stop=last, skip_group_check=True)

        def phase_d(ci):
            c0, cw = CHUNKS[ci]
            lnq = pp.tile([128, cw, O], F32, tag=f"lnq{ci % 2}",
                          name=f"{r}lnq{ci}")
            nc.scalar.activation(lnq[:], ps_q[ci][:], AF.Ln, scale=-1.0,
                                 bias=clnb[:])
            rr = pp.tile([128, cw, O], F32, tag=f"rr{ci % 2}",
                         name=f"{r}rr{ci}")
            nc.scalar.activation(rr[:], lnq[:], AF.Exp, scale=-0.5)
            nc.vector.scalar_tensor_tensor(
                out=osb[:, c0:c0 + cw, :], in0=ps_s[ci][:],
                scalar=1.0 / 63.0, in1=rr[:], op0=OP.mult, op1=OP.mult)
            s2 = pp.tile([128, cw, O - 1], F32, tag=f"s2{ci % 2}",
                         name=f"{r}s2{ci}")
            nc.scalar.activation(s2[:], osb[:, c0:c0 + cw, 1:O], AF.Square)
            red = pp.tile([128, cw], F32, tag=f"red{ci % 2}",
                          name=f"{r}red{ci}")
            nc.vector.tensor_reduce(red[:], s2[:], axis=mybir.AxisListType.X,
                                    op=OP.add)
            ln0 = pp.tile([128, cw], F32, tag=f"ln0{ci % 2}",
                          name=f"{r}ln0{ci}")
            nc.scalar.activation(ln0[:], red[:], AF.Ln, bias=1.0)
            nc.scalar.activation(osb[:, c0:c0 + cw, 0], ln0[:], AF.Exp,
                                 scale=0.5)
            oview = out_ext[128 * c0:128 * (c0 + cw), :].rearrange(
                "(t p) c -> p t c", p=128)
            eng = nc.sync if ci % 2 == 0 else nc.scalar
            eng.dma_start(out=oview[:, :, 1:O], in_=osb[:, c0:c0 + cw, 1:O])
            eng.dma_start(out=oview[:, :, 0:1], in_=osb[:, c0:c0 + cw, 0:1])

        box_all()
        phase_d(0)
        phase_d(1)
        phase_d(2)
        phase_d(3)


_NC_CACHE = None


def _get_nc():
    global _NC_CACHE
    if _NC_CACHE is None:
        _NC_CACHE = build_nc()
    return _NC_CACHE


def host_consts(kernels):
    # u = -l_inner(x,k) = x0*k0 - sum_{c>=1} x_c*k_c ; col O is sum_{c>=1} x_c
    gk_ext = np.zeros((C, O + 1), dtype=np.float32)
    gk_ext[:, :O] = kernels.astype(np.float32).T
    gk_ext[1:, :O] *= -1.0
    gk_ext[1:, O] = 1.0
    return gk_ext


def pad_image(img):
    """[56,56,64] -> host-padded [NT1*128, 64] on the 58x58 grid."""
    xp = np.zeros((NT1 * 128, C), dtype=np.float32)
    grid = xp[:GW * GW].reshape(GW, GW, C)
    grid[1:57, 1:57] = img
    return xp


def unpad_out(o):
    """[NP,64] padded field -> [56,56,64] interior."""
    return o[:GW * GW].reshape(GW, GW, O)[1:57, 1:57]


def core_inputs(x, kernels, core=0):
    import ml_dtypes
    xp = pad_image(np.asarray(x[core], dtype=np.float32))
    return {
        "x": xp,
        "gk_ext": np.ascontiguousarray(host_consts(kernels)),
        "bands": np.ascontiguousarray(
            BANDS.transpose(1, 0, 2).astype(ml_dtypes.bfloat16)),
        "ident": np.eye(128, dtype=np.float32),
    }


def kernel(x, kernels):
    x = np.asarray(x, dtype=np.float32)
    kernels = np.asarray(kernels, dtype=np.float32)
    B = x.shape[0]
    assert x.shape == (B, H, W, C) and B == 8, x.shape
    nc = _get_nc()
    in_maps = [core_inputs(x, kernels, core=i) for i in range(8)]
    res = run_bass_kernel_spmd(nc, in_maps, core_ids=list(range(8)),
                               trace=bool(int(os.environ.get("KTRACE", "0"))))
    if res.exec_time_ns is not None:
        print(f"HW exec time: {res.exec_time_ns} ns")
    out = np.stack([unpad_out(res.results[i]["out"]) for i in range(8)])
    return out.astype(np.float32)


# revision 21
# speedup vs baseline: 1.1702x; 1.0506x over previous
"""LorentzConv2d Trainium2 kernel v6.

Full-input contract: kernel(x=[8,56,56,64], kernels=[64,64]) -> [8,56,56,64].
Data-parallel over batch: one image per NeuronCore (8 cores).

Per-core algorithm on the zero-padded 58x58 grid, linearized l = 58*gh+gw,
tiled l = 128*t + p (p = partition):
  u[l,o]   = sum_c x[l,c] g_c k[o,c]    (PE, f32; col O accumulates sx)
  D[l,o]   = acosh(u)^2 = ln(u + sqrt(u^2-1+g))^2   (ACT chain per group,
             sqrt via exp(0.5 ln): single act-table set, warmed up front;
             u+rt via a PE identity-matmul accumulate into the u PSUM)
  Q[l,o]   = -box3x3(D^2) + 2 sum_d box_d( D_si * D_sj * G_d )   (PE bands)
  S1[l,o]  = box3x3(sx*D)
  out_o    = (S1/63) * exp(-0.5 ln(-Q)) ; out_0 = exp(0.5 ln(1+sum out_o^2))
G products tg_d live in quad super-tiles so the reduction tree runs one DVE
op per level for 4 deltas at once.  Shift copies ride the SP queue (consts
on Pool's SWDGE); D shifts are half-split so they flow while the dist chain
still runs.  Edge box matmuls are range-clamped (fields carry no pads).
Engine split: DVE = tg muls, trees, 9 deltas' pair muls; Pool = s1 field +
3 late deltas' pair muls; ACT = dist chain + xT copies + diag field.
"""

import os
import numpy as np

import concourse.bass as bass
import concourse.bacc as bacc
import concourse.tile as tile
from concourse import mybir
from concourse.bass_utils import run_bass_kernel_spmd

import concourse.bacc as _bacc_mod
from concourse.hw_specs import get_activation_tables as _orig_gat


def _gat(arch):
    tabs = _orig_gat(arch)
    keep = {"sqrt_and_others", "natural_log_exp_and_others"}
    if keep <= set(tabs):
        return {k: (v if k in keep else set()) for k, v in tabs.items()}
    return tabs


_bacc_mod.get_activation_tables = _gat

F32 = mybir.dt.float32
BF16 = mybir.dt.bfloat16
AF = mybir.ActivationFunctionType
OP = mybir.AluOpType

# geometry
H = W = 56
C = 64
O = 64
GW = 58                  # padded grid width (58x58)
NT = 27                  # 128-row tiles covering 58*58=3364 (+ tail)
NP = NT * 128            # 3456
NT1 = NT + 1             # +1 zero tail tile for shifted reads
SQ_GUARD = 1e-4          # replaces the max(u, 1+eps) clamp inside sqrt

# (dh, dw) per positive window-pair offset d = 58*dh + dw
DELTAS = {1: (0, 1), 2: (0, 2), 56: (1, -2), 57: (1, -1), 58: (1, 0),
          59: (1, 1), 60: (1, 2), 114: (2, -2), 115: (2, -1), 116: (2, 0),
          117: (2, 1), 118: (2, 2)}
# d -> (si, sj) with d = sj - si, both in the copy basis
PAIRS = {1: (1, 2), 2: (0, 2), 56: (2, 58), 57: (1, 58), 58: (0, 58),
         59: (1, 60), 60: (0, 60), 114: (2, 116), 115: (1, 116),
         116: (0, 116), 117: (1, 118), 118: (0, 118)}
XSHIFTS = [2, 58, 60, 116, 118]   # unsigned x copies (xc_s), arrival order
GXSHIFTS = [1, 2]                 # signed (col0-negated) copies (gxc_s)
DSHIFTS = [1, 2, 58, 60, 116, 118]
# delta processing order (by shift-copy arrival); tg quads share super-tiles
DORDER = [2, 1, 58, 57, 56, 60, 59, 116, 118, 117, 115, 114]
POOL_DS = {116, 115, 114}    # deltas whose pair muls run on Pool
# pass order for the PE box matmuls: late/Pool fields last
PORDER = [2, 1, 58, 57, 56, 60, 59, 118, 117, 116, 115, 114]
# dist-chain tile groups (pipelined): psu PSUM tiles are per-group
UGROUPS = [(0, 7), (7, 7), (14, 7), (21, 6)]


def _interval(d):
    return range(max(-1, -1 - d), min(1, 1 - d) + 1)


def _build_passes():
    box33 = [58 * a + b for a in (-1, 0, 1) for b in (-1, 0, 1)]

    def dpass(d):
        dh, dw = DELTAS[d]
        si, _ = PAIRS[d]
        box = [58 * a + b - si for a in _interval(dh) for b in _interval(dw)]
        return (f"d{d}", d, 2.0, box, "q")

    passes = [dpass(PORDER[0]), dpass(PORDER[1]),
              ("diag", None, -1.0, box33, "q"),
              ("s1", None, 1.0, box33, "s")]
    for d in PORDER[2:]:
        passes.append(dpass(d))
    return passes


def _build_bands(passes):
    """Banded-Toeplitz matrices. T[m, i] = coeff iff the source row m of tile
    c+j supplies out row i:  m = i + t - 128j for t in box.  Side j=0 first
    so the first matmul of every chunk covers the full PSUM tile."""
    mats = []
    sides = []
    for (_, _, coeff, box, _) in passes:
        plist = []
        for j in (0, -1, 1):
            T = np.zeros((128, 128), dtype=np.float32)
            for t in set(box):
                dd = t - 128 * j
                if -127 <= dd <= 127:
                    idx = np.arange(max(0, dd), 128 + min(0, dd))
                    T[idx, idx - dd] = coeff
            if np.any(T):
                plist.append((j, len(mats)))
                mats.append(T)
        sides.append(plist)
    return np.stack(mats), sides


PASSES = _build_passes()
BANDS, PASS_SIDES = _build_bands(PASSES)
NB = BANDS.shape[0]
CHUNKS = [(0, 8), (8, 8), (16, 8), (24, 3)]


def _shift_copy(nc, dst, src, s, eng_a=None, eng_b=None, groups=None):
    """dst[p, 0:NT, :] = src rows l+s (l = 128t+p), via two partition-shifted
    SBUF->SBUF DMAs. src is [128, NT1, inner] with a zero tail tile."""
    eng_a = eng_a or nc.sync
    eng_b = eng_b or eng_a
    assert 0 < s < 128
    if groups is None:
        groups = [(0, NT)]
    for (t0, tn) in groups:
        eng_a.dma_start(out=dst[0:128 - s, t0:t0 + tn, :],
                        in_=src[s:128, t0:t0 + tn, :])
        eng_b.dma_start(out=dst[128 - s:128, t0:t0 + tn, :],
                        in_=src[0:s, t0 + 1:t0 + tn + 1, :])


def _rep2(t, n_inner):
    """[128, NT, 2] tile viewed as [128, NT, n_inner/2, 2] via paired
    stride-1 reads (keeps the DVE 16-bit 2x mode on broadcast multiplies)."""
    return t[:].unsqueeze(2).to_broadcast([128, NT, n_inner // 2, 2])


def _as4(ap, n_inner):
    """[128, NT, n_inner] AP viewed as [128, NT, n_inner/2, 2]."""
    return ap.rearrange("p t (a b) -> p t a b", b=2)


def build_nc(reps=1):
    nc = bacc.Bacc(None)
    x_in = nc.declare_dram_parameter("x", [NT1 * 128, C], F32, isOutput=False)
    x16_in = nc.declare_dram_parameter("x16", [NT1 * 128, C], BF16,
                                       isOutput=False)
    gk_in = nc.declare_dram_parameter("gk_ext", [C, O + 1], F32,
                                      isOutput=False)
    bands_in = nc.declare_dram_parameter("bands", [128, NB, 128], BF16,
                                         isOutput=False)
    id_in = nc.declare_dram_parameter("ident", [128, 128], F32, isOutput=False)
    out_ext = nc.declare_dram_parameter("out", [NP, O], F32, isOutput=True)

    with tile.TileContext(nc) as tc:
        for rep in range(reps):
            with (
                tc.tile_pool(name=f"sg{rep}", bufs=1) as sg,
                tc.tile_pool(name=f"pp{rep}", bufs=1) as pp,
            ):
                _one_rep(nc, tc, sg, pp, x_in, x16_in, gk_in,
                         bands_in, id_in, out_ext, rep)
    nc.finalize()
    return nc


def _one_rep(nc, tc, sg, pp, x_in, x16_in, gk_in, bands_in, id_in,
             out_ext, rep):
    r = f"r{rep}_"

    def T(shape, dt, name):
        return sg.tile(shape, dt, tag=r + name, name=r + name)

    # ---- consts on Pool's SWDGE (id first: gates transposes)
    id_sb = T([128, 128], F32, "id")
    nc.gpsimd.dma_start(out=id_sb[:], in_=id_in[:])
    gk_sb = T([C, O + 1], F32, "gk")
    nc.gpsimd.dma_start(out=gk_sb[:], in_=gk_in[:])

    # warm the single act-table set (ln/exp) before any ACT work
    clnb = T([128, 1], F32, "clnb")
    nc.gpsimd.memset(clnb[:], 1e-30)
    warm = T([128, 1], F32, "warm")
    nc.scalar.activation(warm[:], clnb[:], AF.Ln)

    # ---- x f32 chunks + x16 on SP: small first chunk gates transposes,
    # x16 right after it so the shift-copy stream starts early
    x_sb = T([128, NT1, C], F32, "x_sb")
    xview = x_in.rearrange("(t p) c -> p t c", p=128)
    XCHUNKS = [(0, 4), (4, 8), (12, 8), (20, 8)]
    nc.sync.dma_start(out=x_sb[:, 0:4, :], in_=xview[:, 0:4, :])
    x16 = T([128, NT1, C], BF16, "x16")
    nc.sync.dma_start(out=x16[:],
                      in_=x16_in.rearrange("(t p) c -> p t c", p=128))
    for (t0, tn) in XCHUNKS[1:]:
        nc.sync.dma_start(out=x_sb[:, t0:t0 + tn, :],
                          in_=xview[:, t0:t0 + tn, :])

    # gx16 (col0-negated x16) on DVE (idle head)
    gx16 = T([128, NT1, C], BF16, "gx16")
    nc.vector.tensor_copy(gx16[:], x16[:])
    nc.vector.tensor_scalar_mul(gx16[:, :, 0], gx16[:, :, 0], -1.0)

    # ---- x/gx shift copies, all on the idle SP queue
    xc = {0: x16}
    gxc = {0: gx16}
    shift_jobs = ([("x", 2), ("gx", 1), ("gx", 2)]
                  + [("x", s) for s in XSHIFTS if s != 2])
    for (kind, s) in shift_jobs:
        src, dstmap = (x16, xc) if kind == "x" else (gx16, gxc)
        dstmap[s] = T([128, NT, C], BF16, f"{kind}c{s}")
        _shift_copy(nc, dstmap[s], src, s, eng_a=nc.sync, eng_b=nc.sync)

    # ---- phase A: per group: transposes + u matmuls, then the dist chain
    d16 = T([128, NT1, O], BF16, "d16")
    nc.vector.memset(d16[:, NT, :], 0.0)
    sx_sb = T([128, NT], F32, "sx")
    cm1g = T([128, 1], F32, "cm1g")
    nc.gpsimd.memset(cm1g[:], -1.0 + SQ_GUARD)
    cmone = T([128, 1], F32, "cmone")
    nc.gpsimd.memset(cmone[:], -1.0)
    sx2 = T([128, NT, 2], BF16, "sx2")

    with (
        tc.tile_pool(name=r + "psA", bufs=1, space="PSUM") as psA,
        tc.tile_pool(name=r + "psT", bufs=3, space="PSUM") as psT,
    ):
        xT = T([64, NT, 128], F32, "xT")
        id16 = T([128, 128], BF16, "id16")
        nc.vector.tensor_copy(id16[:], id_sb[:])
        psu_g = [psA.tile([128, 7, O + 1], F32, tag=f"{r}psu{i}",
                          name=f"{r}psu{i}") for i in range(4)]

        def chain(gi):
            t0, tn = UGROUPS[gi]
            b = f"g{gi % 2}_"
            u_ps = psu_g[gi][:, :tn, 0:O]
            sq = pp.tile([128, 7, O], F32, tag=b + "sq", name=f"{r}sq{gi}")
            nc.scalar.activation(sq[:, :tn, :], u_ps, AF.Square)
            nc.scalar.copy(sx_sb[:, t0:t0 + tn], psu_g[gi][:, :tn, O])
            rl1 = pp.tile([128, 7, O], F32, tag=b + "rl1", name=f"{r}rl1{gi}")
            nc.scalar.activation(rl1[:, :tn, :], sq[:, :tn, :], AF.Relu,
                                 bias=cm1g[:])
            lnr = pp.tile([128, 7, O], F32, tag=b + "sq", name=f"{r}lnr{gi}")
            nc.scalar.activation(lnr[:, :tn, :], rl1[:, :tn, :], AF.Ln,
                                 bias=clnb[:])
            rt = pp.tile([128, 7, O], BF16, tag=b + "rt16", name=f"{r}rt{gi}")
            nc.scalar.activation(rt[:, :tn, :], lnr[:, :tn, :], AF.Exp,
                                 scale=0.5)
            nc.tensor.matmul(u_ps, id16[:], rt[:, :tn, :],
                             start=False, stop=True, skip_group_check=True)
            rl = pp.tile([128, 7, O], F32, tag=b + "rl1", name=f"{r}rl{gi}")
            nc.scalar.activation(rl[:, :tn, :], u_ps, AF.Relu,
                                 bias=cmone[:])
            lnv = pp.tile([128, 7, O], F32, tag=b + "sq", name=f"{r}lnv{gi}")
            nc.scalar.activation(lnv[:, :tn, :], rl[:, :tn, :], AF.Ln,
                                 bias=1.0)
            nc.scalar.activation(d16[:, t0:t0 + tn, :], lnv[:, :tn, :],
                                 AF.Square)

        for gi, (t0, tn) in enumerate(UGROUPS):
            for i in range(tn):
                tl = t0 + i
                xt_ps = psT.tile([C, 128], F32)
                nc.tensor.transpose(xt_ps[:], x_sb[:, tl, :], id_sb[:])
                nc.scalar.copy(xT[:, tl, :], xt_ps[:])
                nc.tensor.matmul(psu_g[gi][:, i, :], xT[:, tl, :], gk_sb[:],
                                 start=True, stop=True)
            chain(gi)
        nc.gpsimd.tensor_copy(sx2[:], sx_sb[:].unsqueeze(2).to_broadcast(
            [128, NT, 2]))

    # ---- shifted D copies, half-split so they flow during the chain
    HALVES = [(0, 14), (14, 13)]
    dc = {0: d16}
    for s in DSHIFTS:
        dc[s] = T([128, NT, O], BF16, f"dc{s}")
        _shift_copy(nc, dc[s], d16, s, eng_a=nc.sync, eng_b=nc.sync,
                    groups=HALVES)

    # bands load late on SWDGE: needed only when the box matmuls start
    bands_sb = T([128, NB, 128], BF16, "bands")
    nc.gpsimd.dma_start(out=bands_sb[:], in_=bands_in[:])

    # ---- fields ([128, NT, O], no pads: edge matmuls are range-clamped)
    fields = {}

    def new_field(key):
        f = sg.tile([128, NT, O], BF16, tag=f"{r}f{key}", name=f"{r}f{key}")
        fields[key] = f
        return f

    # tg pair super-tiles: one DVE op per tree level covers 2 deltas.
    # Rotating pp tags: at most 2 pairs in flight.
    NPAIR = len(DORDER) // 2
    tgq = [pp.tile([128, NT, 2 * C], BF16, tag=f"tgq{q % 2}",
                   name=f"{r}tgq{q}") for q in range(NPAIR)]
    trq = [[pp.tile([128, NT, 2 * w], BF16, tag=f"trq{q % 2}_{w}",
                    name=f"{r}trq{q}_{w}")
            for w in (32, 16, 8, 4, 2)] for q in range(NPAIR)]
    g2 = {}

    def emit_pair(q):
        """tg muls for the pair's 2 deltas, then one batched tree."""
        ds = DORDER[2 * q:2 * q + 2]
        for k, d in enumerate(ds):
            si, sj = PAIRS[d]
            nc.vector.tensor_mul(tgq[q][:, :, 64 * k:64 * k + 64],
                                 gxc[si][:, 0:NT, :], xc[sj][:, 0:NT, :])
        src = tgq[q][:].rearrange("p t (k c) -> p t k c", k=2)
        w = C // 2
        for lvl in range(5):
            dstt = trq[q][lvl][:].rearrange("p t (k c) -> p t k c", k=2)
            with nc.allow_low_precision(reason="bf16 tree partials"):
                nc.vector.tensor_add(dstt, src[:, :, :, 0:w],
                                     src[:, :, :, w:2 * w])
            src = dstt
            w //= 2
        for k, d in enumerate(ds):
            g = T([128, NT, 2], BF16, f"g{d}")
            tt = trq[q][4]
            with nc.allow_low_precision(reason="bf16 G"):
                nc.vector.tensor_add(
                    g[:], tt[:, :, 2 * k:2 * k + 1].to_broadcast([128, NT, 2]),
                    tt[:, :, 2 * k + 1:2 * k + 2].to_broadcast([128, NT, 2]))
            g2[d] = g

    # the last four fields reuse dead xc buffers (tag aliasing; the tile
    # framework serializes the write after the buffer's final tg read)
    FIELD_ALIAS = {116: "xc58", 115: "xc60", 114: "xc116", 117: "xc118"}

    def new_field2(key, d):
        if d in FIELD_ALIAS:
            f = sg.tile([128, NT, O], BF16, tag=r + FIELD_ALIAS[d],
                        name=f"{r}f{key}")
            fields[key] = f
            return f
        return new_field(key)

    def pair_muls(d, i):
        si, sj = PAIRS[d]
        f = new_field2(f"d{d}", d)
        if d in POOL_DS:
            t2 = sg.tile([128, NT, O], BF16, tag=f"{r}t2p{d}",
                         name=f"{r}t2p{d}")
            nc.gpsimd.tensor_mul(t2[:], dc[si][:, 0:NT, :],
                                 dc[sj][:, 0:NT, :])
            with nc.allow_low_precision(reason="bf16 field"):
                nc.gpsimd.tensor_mul(
                    f[:], t2[:], g2[d][:, :, 0:1].to_broadcast([128, NT, O]))
        else:
            t2 = pp.tile([128, NT, O], BF16, tag=f"t2{i % 2}",
                         name=f"{r}t2{d}")
            nc.vector.tensor_mul(t2[:], dc[si][:, 0:NT, :],
                                 dc[sj][:, 0:NT, :])
            nc.vector.tensor_mul(_as4(f[:], O), _as4(t2[:], O),
                                 _rep2(g2[d], O))

    emit_pair(0)
    for i, d in enumerate(DORDER):
        if i % 2 == 0 and i // 2 + 1 < NPAIR:
            emit_pair(i // 2 + 1)
        pair_muls(d, i)
        if i == 1:
            fdiag = new_field("diag")
            nc.scalar.activation(fdiag[:], d16[:, 0:NT, :], AF.Square)
            fs1 = new_field("s1")
            nc.gpsimd.tensor_mul(
                fs1[:], d16[:, 0:NT, :],
                sx2[:, :, 0:1].to_broadcast([128, NT, O]))

    # ---- pass-major banded box matmuls over all 4 chunks; edges clamped
    osb = T([128, NT, O], F32, "osb")
    with (
        tc.tile_pool(name=r + "psQ", bufs=1, space="PSUM") as psQ,
        tc.tile_pool(name=r + "psS", bufs=1, space="PSUM") as psS,
    ):
        ps_q = [psQ.tile([128, cw, O], F32, tag=f"{r}psq{ci}",
                         name=f"{r}psq{ci}") for ci, (c0, cw) in
                enumerate(CHUNKS)]
        ps_s = [psS.tile([128, cw, O], F32, tag=f"{r}pss{ci}",
                         name=f"{r}pss{ci}") for ci, (c0, cw) in
                enumerate(CHUNKS)]

        def _emit_count(tgt_kind, ci):
            c0, cw = CHUNKS[ci]
            n = 0
            for pi, p in enumerate(PASSES):
                if p[4] != tgt_kind:
                    continue
                for (j, _) in PASS_SIDES[pi]:
                    if min(NT, c0 + j + cw) - max(0, c0 + j) > 0:
                        n += 1
            return n

        nq_ = [_emit_count("q", ci) for ci in range(4)]
        ns_ = [_emit_count("s", ci) for ci in range(4)]
        wq = [0] * len(CHUNKS)
        ws = [0] * len(CHUNKS)

        def box_all():
            for pi, (pname, dkey, coeff, box, tgt_kind) in enumerate(PASSES):
                fkey = "diag" if pname == "diag" else (
                    "s1" if pname == "s1" else f"d{dkey}")
                f = fields[fkey]
                for (j, bi) in PASS_SIDES[pi]:
                    for ci in range(4):
                        c0, cw = CHUNKS[ci]
                        s0 = max(0, c0 + j)
                        s1 = min(NT, c0 + j + cw)
                        if s1 <= s0:
                            continue
                        oo = s0 - (c0 + j)
                        if tgt_kind == "q":
                            tgt, first, last = ps_q[ci], wq[ci] == 0, \
                                wq[ci] == nq_[ci] - 1
                            wq[ci] += 1
                        else:
                            tgt, first, last = ps_s[ci], ws[ci] == 0, \
                                ws[ci] == ns_[ci] - 1
                            ws[ci] += 1
                        nc.tensor.matmul(
                            tgt[:, oo:oo + (s1 - s0), :],
                            bands_sb[:, bi, :], f[:, s0:s1, :],
                            start=first, stop=last, skip_group_check=True)

        def phase_d(ci):
            c0, cw = CHUNKS[ci]
            lnq = pp.tile([128, cw, O], F32, tag=f"lnq{ci % 2}",
                          name=f"{r}lnq{ci}")
            nc.scalar.activation(lnq[:], ps_q[ci][:], AF.Ln, scale=-1.0,
                                 bias=clnb[:])
            rr = pp.tile([128, cw, O], F32, tag=f"rr{ci % 2}",
                         name=f"{r}rr{ci}")
            nc.scalar.activation(rr[:], lnq[:], AF.Exp, scale=-0.5)
            nc.vector.scalar_tensor_tensor(
                out=osb[:, c0:c0 + cw, :], in0=ps_s[ci][:],
                scalar=1.0 / 63.0, in1=rr[:], op0=OP.mult, op1=OP.mult)
            s2 = pp.tile([128, cw, O - 1], F32, tag=f"s2{ci % 2}",
                         name=f"{r}s2{ci}")
            nc.scalar.activation(s2[:], osb[:, c0:c0 + cw, 1:O], AF.Square)
            red = pp.tile([128, cw], F32, tag=f"red{ci % 2}",
                          name=f"{r}red{ci}")
            nc.vector.tensor_reduce(red[:], s2[:], axis=mybir.AxisListType.X,
                                    op=OP.add)
            ln0 = pp.tile([128, cw], F32, tag=f"ln0{ci % 2}",
                          name=f"{r}ln0{ci}")
            nc.scalar.activation(ln0[:], red[:], AF.Ln, bias=1.0)
            nc.scalar.activation(osb[:, c0:c0 + cw, 0], ln0[:], AF.Exp,
                                 scale=0.5)
            oview = out_ext[128 * c0:128 * (c0 + cw), :].rearrange(
                "(t p) c -> p t c", p=128)
            eng = nc.sync if ci % 2 == 0 else nc.scalar
            eng.dma_start(out=oview[:, :, 1:O], in_=osb[:, c0:c0 + cw, 1:O])
            eng.dma_start(out=oview[:, :, 0:1], in_=osb[:, c0:c0 + cw, 0:1])

        box_all()
        phase_d(0)
        phase_d(1)
        phase_d(2)
        phase_d(3)


_NC_CACHE = None


def _get_nc():
    global _NC_CACHE
    if _NC_CACHE is None:
        _NC_CACHE = build_nc()
    return _NC_CACHE


def host_consts(kernels):
    # u = -l_inner(x,k) = x0*k0 - sum_{c>=1} x_c*k_c ; col O is sum_{c>=1} x_c
    gk_ext = np.zeros((C, O + 1), dtype=np.float32)
    gk_ext[:, :O] = kernels.astype(np.float32).T
    gk_ext[1:, :O] *= -1.0
    gk_ext[1:, O] = 1.0
    return gk_ext


def pad_image(img):
    """[56,56,64] -> host-padded [NT1*128, 64] on the 58x58 grid."""
    xp = np.zeros((NT1 * 128, C), dtype=np.float32)
    grid = xp[:GW * GW].reshape(GW, GW, C)
    grid[1:57, 1:57] = img
    return xp


def unpad_out(o):
    """[NP,64] padded field -> [56,56,64] interior."""
    return o[:GW * GW].reshape(GW, GW, O)[1:57, 1:57]


def core_inputs(x, kernels, core=0):
    import ml_dtypes
    xp = pad_image(np.asarray(x[core], dtype=np.float32))
    return {
        "x": xp,
        "x16": xp.astype(ml_dtypes.bfloat16),
        "gk_ext": np.ascontiguousarray(host_consts(kernels)),
        "bands": np.ascontiguousarray(
            BANDS.transpose(1, 0, 2).astype(ml_dtypes.bfloat16)),
        "ident": np.eye(128, dtype=np.float32),
    }


def kernel(x, kernels):
    x = np.asarray(x, dtype=np.float32)
    kernels = np.asarray(kernels, dtype=np.float32)
    B = x.shape[0]
    assert x.shape == (B, H, W, C) and B == 8, x.shape
    nc = _get_nc()
    in_maps = [core_inputs(x, kernels, core=i) for i in range(8)]
    res = run_bass_kernel_spmd(nc, in_maps, core_ids=list(range(8)),
                               trace=bool(int(os.environ.get("KTRACE", "0"))))
    if res.exec_time_ns is not None:
        print(f"HW exec time: {res.exec_time_ns} ns")
    out = np.stack([unpad_out(res.results[i]["out"]) for i in range(8)])
    return out.astype(np.float32)


# revision 22
# speedup vs baseline: 1.2070x; 1.0314x over previous
"""LorentzConv2d Trainium2 kernel v7.

Full-input contract: kernel(x=[8,56,56,64], kernels=[64,64]) -> [8,56,56,64].
Data-parallel over batch: one image per NeuronCore (8 cores).

Per-core algorithm on the zero-padded 58x58 grid, linearized l = 58*gh+gw,
tiled l = 128*t + p (p = partition):
  u[l,o]   = sum_c xT[c,l] gk[c,o]      (PE, f32; col O accumulates sx)
  D[l,o]   = acosh(u)^2 = ln(u + sqrt(u^2-1+g))^2   (ACT chain per group,
             sqrt via exp(0.5 ln): single act-table set, warmed up front;
             u+rt via a PE identity-matmul accumulate into the u PSUM)
  Q[l,o]   = -box3x3(D^2) + 2 sum_d box_d( D_si * D_sj * G_d )   (PE bands)
  S1[l,o]  = box3x3(sx*D)
  out_o    = (S1/63) * exp(-0.5 ln(-Q)) ; out_0 = exp(0.5 ln(1+sum out_o^2))
The host supplies xT (transposed x, f32), x16/gx16 (bf16, gx = col0-negated)
and a bf16 identity, so phase A has no transposes or PSUM->SBUF copies: ACT
runs the dist chain as soon as each u group lands.  G products tg_d live in
pair super-tiles (one DVE op per tree level covers 2 deltas).  All shift
copies ride the SP queue (consts on Pool's SWDGE); D shifts are half-split
so they flow while the chain still runs.  Edge box matmuls are
range-clamped (fields carry no pads, and four fields alias dead xc bufs).
Engine split: DVE = tg muls, trees, 8 deltas' pair muls; Pool = s1 field +
4 late deltas' pair muls; ACT = dist chain + diag field.
"""

import os
import numpy as np

import concourse.bass as bass
import concourse.bacc as bacc
import concourse.tile as tile
from concourse import mybir
from concourse.bass_utils import run_bass_kernel_spmd

import concourse.bacc as _bacc_mod
from concourse.hw_specs import get_activation_tables as _orig_gat


def _gat(arch):
    tabs = _orig_gat(arch)
    keep = {"sqrt_and_others", "natural_log_exp_and_others"}
    if keep <= set(tabs):
        return {k: (v if k in keep else set()) for k, v in tabs.items()}
    return tabs


_bacc_mod.get_activation_tables = _gat

F32 = mybir.dt.float32
BF16 = mybir.dt.bfloat16
AF = mybir.ActivationFunctionType
OP = mybir.AluOpType

# geometry
H = W = 56
C = 64
O = 64
GW = 58                  # padded grid width (58x58)
NT = 27                  # 128-row tiles covering 58*58=3364 (+ tail)
NP = NT * 128            # 3456
NT1 = NT + 1             # +1 zero tail tile for shifted reads
SQ_GUARD = 1e-4          # replaces the max(u, 1+eps) clamp inside sqrt

# (dh, dw) per positive window-pair offset d = 58*dh + dw
DELTAS = {1: (0, 1), 2: (0, 2), 56: (1, -2), 57: (1, -1), 58: (1, 0),
          59: (1, 1), 60: (1, 2), 114: (2, -2), 115: (2, -1), 116: (2, 0),
          117: (2, 1), 118: (2, 2)}
# d -> (si, sj) with d = sj - si, both in the copy basis
PAIRS = {1: (1, 2), 2: (0, 2), 56: (2, 58), 57: (1, 58), 58: (0, 58),
         59: (1, 60), 60: (0, 60), 114: (2, 116), 115: (1, 116),
         116: (0, 116), 117: (1, 118), 118: (0, 118)}
XSHIFTS = [2, 58, 60, 116, 118]   # unsigned x copies (xc_s), arrival order
GXSHIFTS = [1, 2]                 # signed (col0-negated) copies (gxc_s)
DSHIFTS = [1, 2, 118, 116, 58, 60]
# delta processing order (by shift-copy arrival)
DORDER = [2, 1, 58, 57, 56, 60, 59, 118, 117, 116, 115, 114]
POOL_DS = {117, 116, 115, 114}   # deltas whose pair muls run on Pool
# pass order for the PE box matmuls: late/Pool fields last
PORDER = [2, 1, 58, 57, 56, 60, 59, 118, 117, 116, 115, 114]
# dist-chain tile groups (pipelined): psu PSUM tiles are per-group
UGROUPS = [(0, 7), (7, 7), (14, 7), (21, 6)]


def _interval(d):
    return range(max(-1, -1 - d), min(1, 1 - d) + 1)


def _build_passes():
    box33 = [58 * a + b for a in (-1, 0, 1) for b in (-1, 0, 1)]

    def dpass(d):
        dh, dw = DELTAS[d]
        si, _ = PAIRS[d]
        box = [58 * a + b - si for a in _interval(dh) for b in _interval(dw)]
        return (f"d{d}", d, 2.0, box, "q")

    passes = [dpass(PORDER[0]), dpass(PORDER[1]),
              ("diag", None, -1.0, box33, "q"),
              ("s1", None, 1.0, box33, "s")]
    for d in PORDER[2:]:
        passes.append(dpass(d))
    return passes


def _build_bands(passes):
    """Banded-Toeplitz matrices. T[m, i] = coeff iff the source row m of tile
    c+j supplies out row i:  m = i + t - 128j for t in box.  Side j=0 first
    so the first matmul of every chunk covers the full PSUM tile."""
    mats = []
    sides = []
    for (_, _, coeff, box, _) in passes:
        plist = []
        for j in (0, -1, 1):
            T = np.zeros((128, 128), dtype=np.float32)
            for t in set(box):
                dd = t - 128 * j
                if -127 <= dd <= 127:
                    idx = np.arange(max(0, dd), 128 + min(0, dd))
                    T[idx, idx - dd] = coeff
            if np.any(T):
                plist.append((j, len(mats)))
                mats.append(T)
        sides.append(plist)
    return np.stack(mats), sides


PASSES = _build_passes()
BANDS, PASS_SIDES = _build_bands(PASSES)
NB = BANDS.shape[0]
CHUNKS = [(0, 8), (8, 8), (16, 8), (24, 3)]


def _shift_copy(nc, dst, src, s, eng_a=None, eng_b=None, groups=None):
    """dst[p, 0:NT, :] = src rows l+s (l = 128t+p), via two partition-shifted
    SBUF->SBUF DMAs. src is [128, NT1, inner] with a zero tail tile."""
    eng_a = eng_a or nc.sync
    eng_b = eng_b or eng_a
    assert 0 < s < 128
    if groups is None:
        groups = [(0, NT)]
    for (t0, tn) in groups:
        eng_a.dma_start(out=dst[0:128 - s, t0:t0 + tn, :],
                        in_=src[s:128, t0:t0 + tn, :])
        eng_b.dma_start(out=dst[128 - s:128, t0:t0 + tn, :],
                        in_=src[0:s, t0 + 1:t0 + tn + 1, :])


def _rep2(t, n_inner):
    """[128, NT, 2] tile viewed as [128, NT, n_inner/2, 2] via paired
    stride-1 reads (keeps the DVE 16-bit 2x mode on broadcast multiplies)."""
    return t[:].unsqueeze(2).to_broadcast([128, NT, n_inner // 2, 2])


def _as4(ap, n_inner):
    """[128, NT, n_inner] AP viewed as [128, NT, n_inner/2, 2]."""
    return ap.rearrange("p t (a b) -> p t a b", b=2)


def build_nc(reps=1):
    nc = bacc.Bacc(None)
    xT_in = nc.declare_dram_parameter("xT", [C, NT1 * 128], F32,
                                      isOutput=False)
    x16_in = nc.declare_dram_parameter("x16", [NT1 * 128, C], BF16,
                                       isOutput=False)
    gx16_in = nc.declare_dram_parameter("gx16", [NT1 * 128, C], BF16,
                                        isOutput=False)
    gk_in = nc.declare_dram_parameter("gk_ext", [C, O + 1], F32,
                                      isOutput=False)
    bands_in = nc.declare_dram_parameter("bands", [128, NB, 128], BF16,
                                         isOutput=False)
    id_in = nc.declare_dram_parameter("ident16", [128, 128], BF16,
                                      isOutput=False)
    out_ext = nc.declare_dram_parameter("out", [NP, O], F32, isOutput=True)

    with tile.TileContext(nc) as tc:
        for rep in range(reps):
            with (
                tc.tile_pool(name=f"sg{rep}", bufs=1) as sg,
                tc.tile_pool(name=f"pp{rep}", bufs=1) as pp,
            ):
                _one_rep(nc, tc, sg, pp, xT_in, x16_in, gx16_in, gk_in,
                         bands_in, id_in, out_ext, rep)
    nc.finalize()
    return nc


def _one_rep(nc, tc, sg, pp, xT_in, x16_in, gx16_in, gk_in, bands_in,
             id_in, out_ext, rep):
    r = f"r{rep}_"

    def T(shape, dt, name):
        return sg.tile(shape, dt, tag=r + name, name=r + name)

    # ---- consts on Pool's SWDGE
    gk_sb = T([C, O + 1], F32, "gk")
    nc.gpsimd.dma_start(out=gk_sb[:], in_=gk_in[:])
    id16 = T([128, 128], BF16, "id16")
    nc.gpsimd.dma_start(out=id16[:], in_=id_in[:])

    # warm the single act-table set (ln/exp) before any ACT work
    clnb = T([128, 1], F32, "clnb")
    nc.gpsimd.memset(clnb[:], 1e-30)
    warm = T([128, 1], F32, "warm")
    nc.scalar.activation(warm[:], clnb[:], AF.Ln)

    # ---- xT chunks (small first: gates the u matmuls), then x16/gx16
    xT = T([64, NT1, 128], F32, "xT")
    xTview = xT_in.rearrange("c (t p) -> c t p", p=128)
    XCHUNKS = [(0, 4), (4, 8), (12, 8), (20, 8)]
    nc.sync.dma_start(out=xT[:, 0:4, :], in_=xTview[:, 0:4, :])
    x16 = T([128, NT1, C], BF16, "x16")
    nc.sync.dma_start(out=x16[:],
                      in_=x16_in.rearrange("(t p) c -> p t c", p=128))
    gx16 = T([128, NT1, C], BF16, "gx16")
    nc.sync.dma_start(out=gx16[:],
                      in_=gx16_in.rearrange("(t p) c -> p t c", p=128))
    for (t0, tn) in XCHUNKS[1:]:
        nc.sync.dma_start(out=xT[:, t0:t0 + tn, :],
                          in_=xTview[:, t0:t0 + tn, :])

    # ---- x/gx shift copies, all on the idle SP queue
    xc = {0: x16}
    gxc = {0: gx16}
    shift_jobs = ([("x", 2), ("gx", 1), ("gx", 2)]
                  + [("x", s) for s in XSHIFTS if s != 2])
    for (kind, s) in shift_jobs:
        src, dstmap = (x16, xc) if kind == "x" else (gx16, gxc)
        dstmap[s] = T([128, NT, C], BF16, f"{kind}c{s}")
        _shift_copy(nc, dstmap[s], src, s, eng_a=nc.sync, eng_b=nc.sync)

    # ---- phase A: per group: u matmuls, then the dist chain
    d16 = T([128, NT1, O], BF16, "d16")
    nc.vector.memset(d16[:, NT, :], 0.0)
    sx_sb = T([128, NT], F32, "sx")
    cm1g = T([128, 1], F32, "cm1g")
    nc.gpsimd.memset(cm1g[:], -1.0 + SQ_GUARD)
    cmone = T([128, 1], F32, "cmone")
    nc.gpsimd.memset(cmone[:], -1.0)
    sx2 = T([128, NT, 2], BF16, "sx2")

    with tc.tile_pool(name=r + "psA", bufs=1, space="PSUM") as psA:
        psu_g = [psA.tile([128, 7, O + 1], F32, tag=f"{r}psu{i}",
                          name=f"{r}psu{i}") for i in range(4)]

        def chain(gi):
            t0, tn = UGROUPS[gi]
            b = f"g{gi % 2}_"
            u_ps = psu_g[gi][:, :tn, 0:O]
            sq = pp.tile([128, 7, O], F32, tag=b + "sq", name=f"{r}sq{gi}")
            nc.scalar.activation(sq[:, :tn, :], u_ps, AF.Square)
            nc.scalar.copy(sx_sb[:, t0:t0 + tn], psu_g[gi][:, :tn, O])
            rl1 = pp.tile([128, 7, O], F32, tag=b + "rl1", name=f"{r}rl1{gi}")
            nc.scalar.activation(rl1[:, :tn, :], sq[:, :tn, :], AF.Relu,
                                 bias=cm1g[:])
            lnr = pp.tile([128, 7, O], F32, tag=b + "sq", name=f"{r}lnr{gi}")
            nc.scalar.activation(lnr[:, :tn, :], rl1[:, :tn, :], AF.Ln,
                                 bias=clnb[:])
            rt = pp.tile([128, 7, O], BF16, tag=b + "rt16", name=f"{r}rt{gi}")
            nc.scalar.activation(rt[:, :tn, :], lnr[:, :tn, :], AF.Exp,
                                 scale=0.5)
            nc.tensor.matmul(u_ps, id16[:], rt[:, :tn, :],
                             start=False, stop=True, skip_group_check=True)
            rl = pp.tile([128, 7, O], F32, tag=b + "rl1", name=f"{r}rl{gi}")
            nc.scalar.activation(rl[:, :tn, :], u_ps, AF.Relu,
                                 bias=cmone[:])
            lnv = pp.tile([128, 7, O], F32, tag=b + "sq", name=f"{r}lnv{gi}")
            nc.scalar.activation(lnv[:, :tn, :], rl[:, :tn, :], AF.Ln,
                                 bias=1.0)
            nc.scalar.activation(d16[:, t0:t0 + tn, :], lnv[:, :tn, :],
                                 AF.Square)

        for gi, (t0, tn) in enumerate(UGROUPS):
            for i in range(tn):
                tl = t0 + i
                nc.tensor.matmul(psu_g[gi][:, i, :], xT[:, tl, :], gk_sb[:],
                                 start=True, stop=True)
            chain(gi)
        nc.gpsimd.tensor_copy(sx2[:], sx_sb[:].unsqueeze(2).to_broadcast(
            [128, NT, 2]))

    # ---- shifted D copies, half-split so they flow during the chain
    HALVES = [(0, 14), (14, 13)]
    dc = {0: d16}
    for s in DSHIFTS:
        dc[s] = T([128, NT, O], BF16, f"dc{s}")
        _shift_copy(nc, dc[s], d16, s, eng_a=nc.sync, eng_b=nc.sync,
                    groups=HALVES)

    # bands load late on SWDGE: needed only when the box matmuls start
    bands_sb = T([128, NB, 128], BF16, "bands")
    nc.gpsimd.dma_start(out=bands_sb[:], in_=bands_in[:])

    # ---- fields ([128, NT, O], no pads: edge matmuls are range-clamped)
    fields = {}

    def new_field(key):
        f = sg.tile([128, NT, O], BF16, tag=f"{r}f{key}", name=f"{r}f{key}")
        fields[key] = f
        return f

    # tg pair super-tiles: one DVE op per tree level covers 2 deltas.
    NPAIR = len(DORDER) // 2
    tgq = [pp.tile([128, NT, 2 * C], BF16, tag=f"tgq{q % 2}",
                   name=f"{r}tgq{q}") for q in range(NPAIR)]
    trq = [[pp.tile([128, NT, 2 * w], BF16, tag=f"trq{q % 2}_{w}",
                    name=f"{r}trq{q}_{w}")
            for w in (32, 16, 8, 4, 2)] for q in range(NPAIR)]
    g2 = {}

    def emit_pair(q):
        """tg muls for the pair's 2 deltas, then one batched tree."""
        ds = DORDER[2 * q:2 * q + 2]
        for k, d in enumerate(ds):
            si, sj = PAIRS[d]
            nc.vector.tensor_mul(tgq[q][:, :, 64 * k:64 * k + 64],
                                 gxc[si][:, 0:NT, :], xc[sj][:, 0:NT, :])
        src = tgq[q][:].rearrange("p t (k c) -> p t k c", k=2)
        w = C // 2
        for lvl in range(5):
            dstt = trq[q][lvl][:].rearrange("p t (k c) -> p t k c", k=2)
            with nc.allow_low_precision(reason="bf16 tree partials"):
                nc.vector.tensor_add(dstt, src[:, :, :, 0:w],
                                     src[:, :, :, w:2 * w])
            src = dstt
            w //= 2
        for k, d in enumerate(ds):
            g = T([128, NT, 2], BF16, f"g{d}")
            tt = trq[q][4]
            with nc.allow_low_precision(reason="bf16 G"):
                nc.vector.tensor_add(
                    g[:], tt[:, :, 2 * k:2 * k + 1].to_broadcast([128, NT, 2]),
                    tt[:, :, 2 * k + 1:2 * k + 2].to_broadcast([128, NT, 2]))
            g2[d] = g

    # the last four fields reuse dead xc buffers (tag aliasing; the tile
    # framework serializes the write after the buffer's final tg read)
    FIELD_ALIAS = {116: "xc58", 115: "xc60", 114: "xc116", 117: "xc118"}

    def new_field2(key, d):
        if d in FIELD_ALIAS:
            f = sg.tile([128, NT, O], BF16, tag=r + FIELD_ALIAS[d],
                        name=f"{r}f{key}")
            fields[key] = f
            return f
        return new_field(key)

    def pair_muls(d, i):
        si, sj = PAIRS[d]
        f = new_field2(f"d{d}", d)
        if d in POOL_DS:
            t2 = sg.tile([128, NT, O], BF16, tag=f"{r}t2p{d}",
                         name=f"{r}t2p{d}")
            nc.gpsimd.tensor_mul(t2[:], dc[si][:, 0:NT, :],
                                 dc[sj][:, 0:NT, :])
            with nc.allow_low_precision(reason="bf16 field"):
                nc.gpsimd.tensor_mul(
                    f[:], t2[:], g2[d][:, :, 0:1].to_broadcast([128, NT, O]))
        else:
            t2 = pp.tile([128, NT, O], BF16, tag=f"t2{i % 2}",
                         name=f"{r}t2{d}")
            nc.vector.tensor_mul(t2[:], dc[si][:, 0:NT, :],
                                 dc[sj][:, 0:NT, :])
            nc.vector.tensor_mul(_as4(f[:], O), _as4(t2[:], O),
                                 _rep2(g2[d], O))

    emit_pair(0)
    for i, d in enumerate(DORDER):
        if i % 2 == 0 and i // 2 + 1 < NPAIR:
            emit_pair(i // 2 + 1)
        pair_muls(d, i)
        if i == 1:
            fdiag = new_field("diag")
            nc.scalar.activation(fdiag[:], d16[:, 0:NT, :], AF.Square)
            fs1 = new_field("s1")
            nc.gpsimd.tensor_mul(
                fs1[:], d16[:, 0:NT, :],
                sx2[:, :, 0:1].to_broadcast([128, NT, O]))

    # ---- pass-major banded box matmuls over all 4 chunks; edges clamped
    osb = T([128, NT, O], F32, "osb")
    with (
        tc.tile_pool(name=r + "psQ", bufs=1, space="PSUM") as psQ,
        tc.tile_pool(name=r + "psS", bufs=1, space="PSUM") as psS,
    ):
        ps_q = [psQ.tile([128, cw, O], F32, tag=f"{r}psq{ci}",
                         name=f"{r}psq{ci}") for ci, (c0, cw) in
                enumerate(CHUNKS)]
        ps_s = [psS.tile([128, cw, O], F32, tag=f"{r}pss{ci}",
                         name=f"{r}pss{ci}") for ci, (c0, cw) in
                enumerate(CHUNKS)]

        def _emit_count(tgt_kind, ci):
            c0, cw = CHUNKS[ci]
            n = 0
            for pi, p in enumerate(PASSES):
                if p[4] != tgt_kind:
                    continue
                for (j, _) in PASS_SIDES[pi]:
                    if min(NT, c0 + j + cw) - max(0, c0 + j) > 0:
                        n += 1
            return n

        nq_ = [_emit_count("q", ci) for ci in range(4)]
        ns_ = [_emit_count("s", ci) for ci in range(4)]
        wq = [0] * len(CHUNKS)
        ws = [0] * len(CHUNKS)

        def box_all():
            for pi, (pname, dkey, coeff, box, tgt_kind) in enumerate(PASSES):
                fkey = "diag" if pname == "diag" else (
                    "s1" if pname == "s1" else f"d{dkey}")
                f = fields[fkey]
                for (j, bi) in PASS_SIDES[pi]:
                    for ci in range(4):
                        c0, cw = CHUNKS[ci]
                        s0 = max(0, c0 + j)
                        s1 = min(NT, c0 + j + cw)
                        if s1 <= s0:
                            continue
                        oo = s0 - (c0 + j)
                        if tgt_kind == "q":
                            tgt, first, last = ps_q[ci], wq[ci] == 0, \
                                wq[ci] == nq_[ci] - 1
                            wq[ci] += 1
                        else:
                            tgt, first, last = ps_s[ci], ws[ci] == 0, \
                                ws[ci] == ns_[ci] - 1
                            ws[ci] += 1
                        nc.tensor.matmul(
                            tgt[:, oo:oo + (s1 - s0), :],
                            bands_sb[:, bi, :], f[:, s0:s1, :],
                            start=first, stop=last, skip_group_check=True)

        def phase_d(ci):
            c0, cw = CHUNKS[ci]
            lnq = pp.tile([128, cw, O], F32, tag=f"lnq{ci % 2}",
                          name=f"{r}lnq{ci}")
            nc.scalar.activation(lnq[:], ps_q[ci][:], AF.Ln, scale=-1.0,
                                 bias=clnb[:])
            rr = pp.tile([128, cw, O], F32, tag=f"rr{ci % 2}",
                         name=f"{r}rr{ci}")
            nc.scalar.activation(rr[:], lnq[:], AF.Exp, scale=-0.5)
            nc.vector.scalar_tensor_tensor(
                out=osb[:, c0:c0 + cw, :], in0=ps_s[ci][:],
                scalar=1.0 / 63.0, in1=rr[:], op0=OP.mult, op1=OP.mult)
            s2 = pp.tile([128, cw, O - 1], F32, tag=f"s2{ci % 2}",
                         name=f"{r}s2{ci}")
            nc.scalar.activation(s2[:], osb[:, c0:c0 + cw, 1:O], AF.Square)
            red = pp.tile([128, cw], F32, tag=f"red{ci % 2}",
                          name=f"{r}red{ci}")
            nc.vector.tensor_reduce(red[:], s2[:], axis=mybir.AxisListType.X,
                                    op=OP.add)
            ln0 = pp.tile([128, cw], F32, tag=f"ln0{ci % 2}",
                          name=f"{r}ln0{ci}")
            nc.scalar.activation(ln0[:], red[:], AF.Ln, bias=1.0)
            nc.scalar.activation(osb[:, c0:c0 + cw, 0], ln0[:], AF.Exp,
                                 scale=0.5)
            oview = out_ext[128 * c0:128 * (c0 + cw), :].rearrange(
                "(t p) c -> p t c", p=128)
            eng = nc.sync if ci % 2 == 0 else nc.scalar
            eng.dma_start(out=oview[:, :, 1:O], in_=osb[:, c0:c0 + cw, 1:O])
            eng.dma_start(out=oview[:, :, 0:1], in_=osb[:, c0:c0 + cw, 0:1])

        box_all()
        phase_d(0)
        phase_d(1)
        phase_d(2)
        phase_d(3)


_NC_CACHE = None


def _get_nc():
    global _NC_CACHE
    if _NC_CACHE is None:
        _NC_CACHE = build_nc()
    return _NC_CACHE


def host_consts(kernels):
    # u = -l_inner(x,k) = x0*k0 - sum_{c>=1} x_c*k_c ; col O is sum_{c>=1} x_c
    gk_ext = np.zeros((C, O + 1), dtype=np.float32)
    gk_ext[:, :O] = kernels.astype(np.float32).T
    gk_ext[1:, :O] *= -1.0
    gk_ext[1:, O] = 1.0
    return gk_ext


def pad_image(img):
    """[56,56,64] -> host-padded [NT1*128, 64] on the 58x58 grid."""
    xp = np.zeros((NT1 * 128, C), dtype=np.float32)
    grid = xp[:GW * GW].reshape(GW, GW, C)
    grid[1:57, 1:57] = img
    return xp


def unpad_out(o):
    """[NP,64] padded field -> [56,56,64] interior."""
    return o[:GW * GW].reshape(GW, GW, O)[1:57, 1:57]


def core_inputs(x, kernels, core=0):
    import ml_dtypes
    xp = pad_image(np.asarray(x[core], dtype=np.float32))
    x16 = xp.astype(ml_dtypes.bfloat16)
    gx16 = x16.copy()
    gx16[:, 0] = -gx16[:, 0]
    return {
        "xT": np.ascontiguousarray(xp.T),
        "x16": x16,
        "gx16": gx16,
        "gk_ext": np.ascontiguousarray(host_consts(kernels)),
        "bands": np.ascontiguousarray(
            BANDS.transpose(1, 0, 2).astype(ml_dtypes.bfloat16)),
        "ident16": np.eye(128, dtype=np.float32).astype(ml_dtypes.bfloat16),
    }


def kernel(x, kernels):
    x = np.asarray(x, dtype=np.float32)
    kernels = np.asarray(kernels, dtype=np.float32)
    B = x.shape[0]
    assert x.shape == (B, H, W, C) and B == 8, x.shape
    nc = _get_nc()
    in_maps = [core_inputs(x, kernels, core=i) for i in range(8)]
    res = run_bass_kernel_spmd(nc, in_maps, core_ids=list(range(8)),
                               trace=bool(int(os.environ.get("KTRACE", "0"))))
    if res.exec_time_ns is not None:
        print(f"HW exec time: {res.exec_time_ns} ns")
    out = np.stack([unpad_out(res.results[i]["out"]) for i in range(8)])
    return out.astype(np.float32)


# revision 43
# speedup vs baseline: 1.3417x; 1.1116x over previous
"""LorentzConv2d Trainium2 kernel v7.

Full-input contract: kernel(x=[8,56,56,64], kernels=[64,64]) -> [8,56,56,64].
Data-parallel over batch: one image per NeuronCore (8 cores).

Per-core algorithm on the zero-padded 58x58 grid, linearized l = 58*gh+gw,
tiled l = 128*t + p (p = partition):
  u[l,o]   = sum_c xT[c,l] gk[c,o]      (PE, f32; col O accumulates sx)
  D[l,o]   = acosh(u)^2 = ln(u + sqrt(u^2-1+g))^2   (ACT chain per group,
             sqrt via exp(0.5 ln): single act-table set, warmed up front;
             u+rt via a PE identity-matmul accumulate into the u PSUM)
  Q[l,o]   = -box3x3(D^2) + 2 sum_d box_d( D_si * D_sj * G_d )   (PE bands)
  S1[l,o]  = box3x3(sx*D)
  out_o    = (S1/63) * exp(-0.5 ln(-Q)) ; out_0 = exp(0.5 ln(1+sum out_o^2))
The host supplies xT (transposed x, f32), x16/gx16 (bf16, gx = col0-negated)
and a bf16 identity, so phase A has no transposes or PSUM->SBUF copies: ACT
runs the dist chain as soon as each u group lands.  G products tg_d live in
pair super-tiles (one DVE op per tree level covers 2 deltas).  All shift
copies ride the SP queue (consts on Pool's SWDGE); D shifts are half-split
so they flow while the chain still runs.  Edge box matmuls are
range-clamped (fields carry no pads, and four fields alias dead xc bufs).
Engine split: DVE = tg muls, trees, 8 deltas' pair muls; Pool = s1 field +
4 late deltas' pair muls; ACT = dist chain + diag field.
"""

import os
import numpy as np

import concourse.bass as bass
import concourse.bacc as bacc
import concourse.tile as tile
from concourse import mybir
from concourse.bass_utils import run_bass_kernel_spmd

import concourse.bacc as _bacc_mod
from concourse.hw_specs import get_activation_tables as _orig_gat


def _gat(arch):
    tabs = _orig_gat(arch)
    keep = {"sqrt_and_others", "natural_log_exp_and_others"}
    if keep <= set(tabs):
        return {k: (v if k in keep else set()) for k, v in tabs.items()}
    return tabs


_bacc_mod.get_activation_tables = _gat

F32 = mybir.dt.float32
BF16 = mybir.dt.bfloat16
AF = mybir.ActivationFunctionType
OP = mybir.AluOpType

# geometry
H = W = 56
C = 64
O = 64
GW = 58                  # padded grid width (58x58)
NT = 27                  # 128-row tiles covering 58*58=3364 (+ tail)
NP = NT * 128            # 3456
NT1 = NT + 1             # +1 zero tail tile for shifted reads
SQ_GUARD = 1e-4          # replaces the max(u, 1+eps) clamp inside sqrt

# (dh, dw) per positive window-pair offset d = 58*dh + dw
DELTAS = {1: (0, 1), 2: (0, 2), 56: (1, -2), 57: (1, -1), 58: (1, 0),
          59: (1, 1), 60: (1, 2), 114: (2, -2), 115: (2, -1), 116: (2, 0),
          117: (2, 1), 118: (2, 2)}
# d -> (si, sj) with d = sj - si, both in the copy basis
PAIRS = {1: (1, 2), 2: (0, 2), 56: (2, 58), 57: (1, 58), 58: (0, 58),
         59: (1, 60), 60: (0, 60), 114: (2, 116), 115: (1, 116),
         116: (0, 116), 117: (1, 118), 118: (0, 118)}
XSHIFTS = [2, 58, 60, 116, 118]   # unsigned x copies (xc_s), arrival order
GXSHIFTS = [1, 2]                 # signed (col0-negated) copies (gxc_s)
DSHIFTS = [1, 2, 118, 116, 58, 60]
# delta processing order (by shift-copy arrival)
DORDER = [2, 1, 118, 117, 116, 115, 114, 58, 57, 56, 60, 59]
POOL_DS = {117, 116, 115, 114}   # deltas whose pair muls run on Pool
# pass order for the PE box matmuls: late/Pool fields last
PORDER = [2, 1, 58, 57, 56, 60, 59, 118, 117, 116, 114, 115]
# dist-chain tile groups (pipelined): psu PSUM tiles are per-group
UGROUPS = [(0, 7), (7, 7), (14, 7), (21, 6)]


def _interval(d):
    return range(max(-1, -1 - d), min(1, 1 - d) + 1)


def _build_passes():
    box33 = [58 * a + b for a in (-1, 0, 1) for b in (-1, 0, 1)]

    def dpass(d):
        dh, dw = DELTAS[d]
        si, _ = PAIRS[d]
        box = [58 * a + b - si for a in _interval(dh) for b in _interval(dw)]
        return (f"d{d}", d, 2.0, box, "q")

    passes = [dpass(PORDER[0]), dpass(PORDER[1]),
              ("diag", None, -1.0, box33, "q"),
              ("s1", None, 1.0, box33, "s")]
    for d in PORDER[2:]:
        passes.append(dpass(d))
    return passes


def _build_bands(passes):
    """Banded-Toeplitz matrices. T[m, i] = coeff iff the source row m of tile
    c+j supplies out row i:  m = i + t - 128j for t in box.  Side j=0 first
    so the first matmul of every chunk covers the full PSUM tile."""
    mats = []
    sides = []
    for (_, _, coeff, box, _) in passes:
        plist = []
        for j in (0, -1, 1):
            T = np.zeros((128, 128), dtype=np.float32)
            for t in set(box):
                dd = t - 128 * j
                if -127 <= dd <= 127:
                    idx = np.arange(max(0, dd), 128 + min(0, dd))
                    T[idx, idx - dd] = coeff
            if np.any(T):
                plist.append((j, len(mats)))
                mats.append(T)
        sides.append(plist)
    return np.stack(mats), sides


PASSES = _build_passes()
BANDS, PASS_SIDES = _build_bands(PASSES)
NB = BANDS.shape[0]
CHUNKS = [(0, 8), (8, 8), (16, 8), (24, 3)]


def _shift_copy(nc, dst, src, s, eng_a=None, eng_b=None, groups=None):
    """dst[p, 0:NT, :] = src rows l+s (l = 128t+p), via two partition-shifted
    SBUF->SBUF DMAs. src is [128, NT1, inner] with a zero tail tile."""
    eng_a = eng_a or nc.sync
    eng_b = eng_b or eng_a
    assert 0 < s < 128
    if groups is None:
        groups = [(0, NT)]
    for (t0, tn) in groups:
        eng_a.dma_start(out=dst[0:128 - s, t0:t0 + tn, :],
                        in_=src[s:128, t0:t0 + tn, :])
        eng_b.dma_start(out=dst[128 - s:128, t0:t0 + tn, :],
                        in_=src[0:s, t0 + 1:t0 + tn + 1, :])


def _rep2(t, n_inner):
    """[128, NT, 2] tile viewed as [128, NT, n_inner/2, 2] via paired
    stride-1 reads (keeps the DVE 16-bit 2x mode on broadcast multiplies)."""
    return t[:].unsqueeze(2).to_broadcast([128, NT, n_inner // 2, 2])


def _as4(ap, n_inner):
    """[128, NT, n_inner] AP viewed as [128, NT, n_inner/2, 2]."""
    return ap.rearrange("p t (a b) -> p t a b", b=2)


def build_nc(reps=1):
    nc = bacc.Bacc(None)
    xT_in = nc.declare_dram_parameter("xT", [C, NT1 * 128], BF16,
                                      isOutput=False)
    # host-pre-shifted bf16 copies: gxc0 (= col0-negated x16) plus the six
    # shifted variants the pair basis needs -- independent HBM loads, so the
    # G-product pipeline starts as soon as each lands (no on-chip shifts)
    shift_ins = {}
    for (kind, s) in [("gx", 0), ("x", 2), ("gx", 1), ("gx", 2),
                      ("x", 58), ("x", 60), ("x", 116), ("x", 118)]:
        shift_ins[(kind, s)] = nc.declare_dram_parameter(
            f"{kind}c{s}", [128, NT * C], BF16, isOutput=False)
    gk_in = nc.declare_dram_parameter("gk_ext", [C, O + 1], BF16,
                                      isOutput=False)
    bands_in = nc.declare_dram_parameter("bands", [128, NB, 128], BF16,
                                         isOutput=False)
    id_in = nc.declare_dram_parameter("ident16", [128, 128], BF16,
                                      isOutput=False)
    out_ext = nc.declare_dram_parameter("out", [NP, O], F32, isOutput=True)

    with tile.TileContext(nc) as tc:
        for rep in range(reps):
            with (
                tc.tile_pool(name=f"sg{rep}", bufs=1) as sg,
                tc.tile_pool(name=f"pp{rep}", bufs=1) as pp,
            ):
                _one_rep(nc, tc, sg, pp, xT_in, shift_ins, gk_in,
                         bands_in, id_in, out_ext, rep)
    nc.finalize()
    return nc


def _one_rep(nc, tc, sg, pp, xT_in, shift_ins, gk_in, bands_in,
             id_in, out_ext, rep):
    r = f"r{rep}_"

    def T(shape, dt, name):
        return sg.tile(shape, dt, tag=r + name, name=r + name)

    # ---- consts on Pool's SWDGE
    gk_sb = T([C, O + 1], BF16, "gk")
    nc.gpsimd.dma_start(out=gk_sb[:], in_=gk_in[:])
    id16 = T([128, 128], BF16, "id16")
    nc.gpsimd.dma_start(out=id16[:], in_=id_in[:])

    # warm the single act-table set (ln/exp) before any ACT work
    clnb = T([128, 1], F32, "clnb")
    nc.gpsimd.memset(clnb[:], 1e-30)
    warm = T([128, 1], F32, "warm")
    nc.scalar.activation(warm[:], clnb[:], AF.Ln)

    # ---- interleave xT chunks with the pre-shifted x/gx loads so the u
    # matmuls AND the first G products both start early
    xT = T([64, NT1, 128], BF16, "xT")
    xTview = xT_in.rearrange("c (t p) -> c t p", p=128)
    xc = {}
    gxc = {}

    def _load_shift(kind, s):
        dstmap = xc if kind == "x" else gxc
        dstmap[s] = T([128, NT, C], BF16, f"{kind}c{s}")
        nc.sync.dma_start(
            out=dstmap[s][:],
            in_=shift_ins[(kind, s)].rearrange("p (t c) -> p t c", c=C))

    nc.sync.dma_start(out=xT[:, 0:7, :], in_=xTview[:, 0:7, :])
    _load_shift("gx", 0)
    _load_shift("x", 2)
    _load_shift("gx", 1)
    nc.sync.dma_start(out=xT[:, 7:14, :], in_=xTview[:, 7:14, :])
    _load_shift("x", 118)
    nc.sync.dma_start(out=xT[:, 14:21, :], in_=xTview[:, 14:21, :])
    _load_shift("gx", 2)
    nc.sync.dma_start(out=xT[:, 21:NT, :], in_=xTview[:, 21:NT, :])
    _load_shift("x", 116)
    _load_shift("x", 58)
    _load_shift("x", 60)

    # ---- phase A: per group: u matmuls, then the dist chain
    d16 = T([128, NT1, O], BF16, "d16")
    nc.vector.memset(d16[:, NT, :], 0.0)
    sx_sb = T([128, NT], F32, "sx")
    cm1g = T([128, 1], F32, "cm1g")
    nc.gpsimd.memset(cm1g[:], -1.0 + SQ_GUARD)
    cmone = T([128, 1], F32, "cmone")
    nc.gpsimd.memset(cmone[:], -1.0)
    sx2 = T([128, NT, 2], BF16, "sx2")

    with tc.tile_pool(name=r + "psA", bufs=1, space="PSUM") as psA:
        psu_g = [psA.tile([128, 7, O + 1], F32, tag=f"{r}psu{i}",
                          name=f"{r}psu{i}") for i in range(4)]
        # hybrid chain: per-group only where PSUM forces it (sq/sx/accum/rl),
        # full-tensor for the SBUF->SBUF middle (fewer, bigger ACT ops)
        bufA = pp.tile([128, NT, O], F32, tag="chA", name=r + "chA")
        bufB = pp.tile([128, NT, O], F32, tag="chB", name=r + "chB")
        rt16 = pp.tile([128, NT, O], BF16, tag="chR", name=r + "chR")

        for gi, (t0, tn) in enumerate(UGROUPS):
            for i in range(tn):
                tl = t0 + i
                nc.tensor.matmul(psu_g[gi][:, i, :], xT[:, tl, :], gk_sb[:],
                                 start=True, stop=True)
            # sq into the full-tensor buffer slice; sx copy
            nc.scalar.activation(bufA[:, t0:t0 + tn, :],
                                 psu_g[gi][:, :tn, 0:O], AF.Square)
            nc.scalar.copy(sx_sb[:, t0:t0 + tn], psu_g[gi][:, :tn, O])
        nc.gpsimd.tensor_copy(sx2[:], sx_sb[:].unsqueeze(2).to_broadcast(
            [128, NT, 2]))
        # the SBUF middle of the chain runs per half (h0 = groups 0-1) so
        # d16 h0 emerges early and the D-shift stream starts sooner
        for (h0, hn, gs) in [(0, 14, (0, 1)), (14, 13, (2, 3))]:
            hs = slice(h0, h0 + hn)
            nc.scalar.activation(bufB[:, hs, :], bufA[:, hs, :], AF.Relu,
                                 bias=cm1g[:])
            nc.scalar.activation(bufA[:, hs, :], bufB[:, hs, :], AF.Ln,
                                 bias=clnb[:])
            nc.scalar.activation(rt16[:, hs, :], bufA[:, hs, :], AF.Exp,
                                 scale=0.5)
            for gi in gs:
                t0, tn = UGROUPS[gi]
                u_ps = psu_g[gi][:, :tn, 0:O]
                nc.tensor.matmul(u_ps, id16[:], rt16[:, t0:t0 + tn, :],
                                 start=False, stop=True,
                                 skip_group_check=True)
                nc.scalar.activation(bufB[:, t0:t0 + tn, :], u_ps, AF.Relu,
                                     bias=cmone[:])
            nc.scalar.activation(bufA[:, hs, :], bufB[:, hs, :], AF.Ln,
                                 bias=1.0)
            nc.scalar.activation(d16[:, hs, :], bufA[:, hs, :], AF.Square)

    # ---- shifted D copies, half-split; all h0 pieces first so every
    # consumer's first half lands as early as possible
    HALVES = [(0, 14), (14, 13)]
    dc = {0: d16}
    for s in DSHIFTS:
        dc[s] = T([128, NT, O], BF16, f"dc{s}")
    for (t0, tn) in HALVES:
        for s in DSHIFTS:
            _shift_copy(nc, dc[s], d16, s, eng_a=nc.sync, eng_b=nc.sync,
                        groups=[(t0, tn)])

    # bands load late on SWDGE: needed only when the box matmuls start
    bands_sb = T([128, NB, 128], BF16, "bands")
    nc.gpsimd.dma_start(out=bands_sb[:], in_=bands_in[:])

    # ---- fields ([128, NT, O], no pads: edge matmuls are range-clamped)
    fields = {}

    def new_field(key):
        f = sg.tile([128, NT, O], BF16, tag=f"{r}f{key}", name=f"{r}f{key}")
        fields[key] = f
        return f

    # tg pair super-tiles: one DVE op per tree level covers 2 deltas.
    NPAIR = len(DORDER) // 2
    tgq = [pp.tile([128, NT, 2 * C], BF16, tag=f"tgq{q % 2}",
                   name=f"{r}tgq{q}") for q in range(NPAIR)]
    trq = [[pp.tile([128, NT, 2 * w], BF16, tag=f"trq{q % 2}_{w}",
                    name=f"{r}trq{q}_{w}")
            for w in (32, 16, 8, 4, 2)] for q in range(NPAIR)]
    g2 = {}

    def emit_pair(q):
        """tg muls for the pair's 2 deltas, then one batched tree."""
        ds = DORDER[2 * q:2 * q + 2]
        for k, d in enumerate(ds):
            si, sj = PAIRS[d]
            nc.vector.tensor_mul(tgq[q][:, :, 64 * k:64 * k + 64],
                                 gxc[si][:, 0:NT, :], xc[sj][:, 0:NT, :])
        src = tgq[q][:].rearrange("p t (k c) -> p t k c", k=2)
        w = C // 2
        for lvl in range(5):
            dstt = trq[q][lvl][:].rearrange("p t (k c) -> p t k c", k=2)
            with nc.allow_low_precision(reason="bf16 tree partials"):
                nc.vector.tensor_add(dstt, src[:, :, :, 0:w],
                                     src[:, :, :, w:2 * w])
            src = dstt
            w //= 2
        for k, d in enumerate(ds):
            g = T([128, NT, 2], BF16, f"g{d}")
            tt = trq[q][4]
            with nc.allow_low_precision(reason="bf16 G"):
                nc.vector.tensor_add(
                    g[:], tt[:, :, 2 * k:2 * k + 1].to_broadcast([128, NT, 2]),
                    tt[:, :, 2 * k + 1:2 * k + 2].to_broadcast([128, NT, 2]))
            g2[d] = g

    # the last four fields reuse dead xc buffers (tag aliasing; the tile
    # framework serializes the write after the buffer's final tg read)
    FIELD_ALIAS = {116: "xc58", 115: "xc60", 114: "xc116", 117: "xc118"}

    def new_field2(key, d):
        if d in FIELD_ALIAS:
            f = sg.tile([128, NT, O], BF16, tag=r + FIELD_ALIAS[d],
                        name=f"{r}f{key}")
            fields[key] = f
            return f
        return new_field(key)

    def pair_muls(d, i):
        si, sj = PAIRS[d]
        f = new_field2(f"d{d}", d)
        if d in POOL_DS:
            # Pool does the t2 halves; DVE applies the G broadcast later
            # (deferred past the DVE stream so it never stalls in-order DVE)
            t2 = sg.tile([128, NT, O], BF16, tag=f"{r}t2p{d}",
                         name=f"{r}t2p{d}")
            for (t0, tn) in HALVES:
                nc.gpsimd.tensor_mul(t2[:, t0:t0 + tn, :],
                                     dc[si][:, t0:t0 + tn, :],
                                     dc[sj][:, t0:t0 + tn, :])
            if d == 114:
                with nc.allow_low_precision(reason="bf16 field"):
                    nc.gpsimd.tensor_mul(
                        f[:], t2[:],
                        g2[d][:, :, 0:1].to_broadcast([128, NT, O]))
            else:
                deferred.append((d, f, t2))
        else:
            t2 = pp.tile([128, NT, O], BF16, tag=f"t2{i % 2}",
                         name=f"{r}t2{d}")
            nc.vector.tensor_mul(t2[:], dc[si][:, 0:NT, :],
                                 dc[sj][:, 0:NT, :])
            nc.vector.tensor_mul(_as4(f[:], O), _as4(t2[:], O),
                                 _rep2(g2[d], O))

    deferred = []
    emit_pair(0)
    for i, d in enumerate(DORDER):
        if i % 2 == 0 and i // 2 + 1 < NPAIR:
            emit_pair(i // 2 + 1)
        pair_muls(d, i)
        if i == 1:
            fdiag = new_field("diag")
            nc.scalar.activation(fdiag[:], d16[:, 0:NT, :], AF.Square)
            fs1 = new_field("s1")
            nc.gpsimd.tensor_mul(
                fs1[:], d16[:, 0:NT, :],
                sx2[:, :, 0:1].to_broadcast([128, NT, O]))
    for (d, f, t2) in deferred:
        nc.vector.tensor_mul(_as4(f[:], O), _as4(t2[:], O),
                             _rep2(g2[d], O))

    # ---- pass-major banded box matmuls over all 4 chunks; edges clamped
    osb = T([128, NT, O], F32, "osb")
    with (
        tc.tile_pool(name=r + "psQ", bufs=1, space="PSUM") as psQ,
        tc.tile_pool(name=r + "psS", bufs=1, space="PSUM") as psS,
    ):
        ps_q = [psQ.tile([128, cw, O], F32, tag=f"{r}psq{ci}",
                         name=f"{r}psq{ci}") for ci, (c0, cw) in
                enumerate(CHUNKS)]
        ps_s = [psS.tile([128, cw, O], F32, tag=f"{r}pss{ci}",
                         name=f"{r}pss{ci}") for ci, (c0, cw) in
                enumerate(CHUNKS)]

        def _emit_count(tgt_kind, ci):
            c0, cw = CHUNKS[ci]
            n = 0
            for pi, p in enumerate(PASSES):
                if p[4] != tgt_kind:
                    continue
                for (j, _) in PASS_SIDES[pi]:
                    if min(NT, c0 + j + cw) - max(0, c0 + j) > 0:
                        n += 1
            return n

        nq_ = [_emit_count("q", ci) for ci in range(4)]
        ns_ = [_emit_count("s", ci) for ci in range(4)]
        wq = [0] * len(CHUNKS)
        ws = [0] * len(CHUNKS)

        def box_all():
            for pi, (pname, dkey, coeff, box, tgt_kind) in enumerate(PASSES):
                fkey = "diag" if pname == "diag" else (
                    "s1" if pname == "s1" else f"d{dkey}")
                f = fields[fkey]
                for (j, bi) in PASS_SIDES[pi]:
                    for ci in range(4):
                        c0, cw = CHUNKS[ci]
                        s0 = max(0, c0 + j)
                        s1 = min(NT, c0 + j + cw)
                        if s1 <= s0:
                            continue
                        oo = s0 - (c0 + j)
                        if tgt_kind == "q":
                            tgt, first, last = ps_q[ci], wq[ci] == 0, \
                                wq[ci] == nq_[ci] - 1
                            wq[ci] += 1
                        else:
                            tgt, first, last = ps_s[ci], ws[ci] == 0, \
                                ws[ci] == ns_[ci] - 1
                            ws[ci] += 1
                        nc.tensor.matmul(
                            tgt[:, oo:oo + (s1 - s0), :],
                            bands_sb[:, bi, :], f[:, s0:s1, :],
                            start=first, stop=last, skip_group_check=True)

        def phase_d(ci):
            c0, cw = CHUNKS[ci]
            lnq = pp.tile([128, cw, O], F32, tag=f"lnq{ci % 2}",
                          name=f"{r}lnq{ci}")
            nc.scalar.activation(lnq[:], ps_q[ci][:], AF.Ln, scale=-1.0,
                                 bias=clnb[:])
            rr = pp.tile([128, cw, O], F32, tag=f"rr{ci % 2}",
                         name=f"{r}rr{ci}")
            nc.scalar.activation(rr[:], lnq[:], AF.Exp, scale=-0.5)
            nc.vector.scalar_tensor_tensor(
                out=osb[:, c0:c0 + cw, :], in0=ps_s[ci][:],
                scalar=1.0 / 63.0, in1=rr[:], op0=OP.mult, op1=OP.mult)
            s2 = pp.tile([128, cw, O - 1], F32, tag=f"s2{ci % 2}",
                         name=f"{r}s2{ci}")
            nc.scalar.activation(s2[:], osb[:, c0:c0 + cw, 1:O], AF.Square)
            red = pp.tile([128, cw], F32, tag=f"red{ci % 2}",
                          name=f"{r}red{ci}")
            nc.vector.tensor_reduce(red[:], s2[:], axis=mybir.AxisListType.X,
                                    op=OP.add)
            ln0 = pp.tile([128, cw], F32, tag=f"ln0{ci % 2}",
                          name=f"{r}ln0{ci}")
            nc.scalar.activation(ln0[:], red[:], AF.Ln, bias=1.0)
            nc.scalar.activation(osb[:, c0:c0 + cw, 0], ln0[:], AF.Exp,
                                 scale=0.5)
            oview = out_ext[128 * c0:128 * (c0 + cw), :].rearrange(
                "(t p) c -> p t c", p=128)
            eng = nc.sync if ci % 2 == 0 else nc.scalar
            eng.dma_start(out=oview[:, :, 1:O], in_=osb[:, c0:c0 + cw, 1:O])
            eng.dma_start(out=oview[:, :, 0:1], in_=osb[:, c0:c0 + cw, 0:1])

        box_all()
        phase_d(0)
        phase_d(1)
        phase_d(2)
        phase_d(3)


_NC_CACHE = None


def _get_nc():
    global _NC_CACHE
    if _NC_CACHE is None:
        _NC_CACHE = build_nc()
    return _NC_CACHE


def host_consts(kernels):
    # u = -l_inner(x,k) = x0*k0 - sum_{c>=1} x_c*k_c ; col O is sum_{c>=1} x_c
    gk_ext = np.zeros((C, O + 1), dtype=np.float32)
    gk_ext[:, :O] = kernels.astype(np.float32).T
    gk_ext[1:, :O] *= -1.0
    gk_ext[1:, O] = 1.0
    return gk_ext


def pad_image(img):
    """[56,56,64] -> host-padded [NT1*128, 64] on the 58x58 grid."""
    xp = np.zeros((NT1 * 128, C), dtype=np.float32)
    grid = xp[:GW * GW].reshape(GW, GW, C)
    grid[1:57, 1:57] = img
    return xp


def unpad_out(o):
    """[NP,64] padded field -> [56,56,64] interior."""
    return o[:GW * GW].reshape(GW, GW, O)[1:57, 1:57]


def core_inputs(x, kernels, core=0):
    import ml_dtypes
    xp = pad_image(np.asarray(x[core], dtype=np.float32))
    x16 = xp.astype(ml_dtypes.bfloat16)
    gx16 = x16.copy()
    gx16[:, 0] = -gx16[:, 0]
    ins = {
        "xT": np.ascontiguousarray(xp.T).astype(ml_dtypes.bfloat16),
        "gk_ext": np.ascontiguousarray(
            host_consts(kernels).astype(ml_dtypes.bfloat16)),
        "bands": np.ascontiguousarray(
            BANDS.transpose(1, 0, 2).astype(ml_dtypes.bfloat16)),
        "ident16": np.eye(128, dtype=np.float32).astype(ml_dtypes.bfloat16),
    }
    for (kind, s) in [("gx", 0), ("x", 2), ("gx", 1), ("gx", 2),
                      ("x", 58), ("x", 60), ("x", 116), ("x", 118)]:
        src = x16 if kind == "x" else gx16
        sh = src[s:s + NT * 128].reshape(NT, 128, C).transpose(1, 0, 2)
        ins[f"{kind}c{s}"] = np.ascontiguousarray(sh).reshape(128, NT * C)
    return ins


def kernel(x, kernels):
    x = np.asarray(x, dtype=np.float32)
    kernels = np.asarray(kernels, dtype=np.float32)
    B = x.shape[0]
    assert x.shape == (B, H, W, C) and B == 8, x.shape
    nc = _get_nc()
    in_maps = [core_inputs(x, kernels, core=i) for i in range(8)]
    res = run_bass_kernel_spmd(nc, in_maps, core_ids=list(range(8)),
                               trace=bool(int(os.environ.get("KTRACE", "0"))))
    if res.exec_time_ns is not None:
        print(f"HW exec time: {res.exec_time_ns} ns")
    out = np.stack([unpad_out(res.results[i]["out"]) for i in range(8)])
    return out.astype(np.float32)
